# revision 38
# baseline (speedup 1.0000x reference)
"""Trainium2 Bass kernel for nn_MemTransformerLM (Transformer-XL layer).

Sharding (8 cores): batch (4) x head-half (2). Core c handles batch b = c//2
and heads [hh*8, hh*8+8), hh = c%2, for all 1024 queries. After o_proj a
2-rank ReduceScatter over core pairs (2b, 2b+1) splits tokens for the FFN:
even core keeps tokens [0,512), odd [512,1024).

v2 rewrite ("exp-split" attention):
 - host pre-transposes x/r -> direct [d-part, klen] loads, no on-chip
   input transposes.
 - softmax split: p = exp(AC) * exp(BD_shifted). Act exponentiates the
   AC and BD PSUM chunks directly into SBUF (no PSUM-drain copies), a
   single gpsimd DMA applies the Transformer-XL rel-shift as a diagonal
   copy, and DVE multiplies the two factors at 2x (all-bf16).
 - the causal/memory mask is a permanently-zeroed tail of the BD buffer
   (exp(-inf) = 0), so masking costs nothing per iteration.
 - PV softmax denominator via a ones-column in V; reciprocal broadcast
   uses gpsimd partition_broadcast instead of a DMA.
 - PV of pair hp-1 is issued at the start of pair hp to keep PE fed.
"""

import contextlib
import math

import numpy as np

import concourse.bass as bass
import concourse.bacc as bacc
import concourse.mybir as mybir
import concourse.tile as tile

F32 = mybir.dt.float32
BF16 = mybir.dt.bfloat16
FP8 = mybir.dt.float8e4
DR = mybir.MatmulPerfMode.DoubleRow
AF = mybir.ActivationFunctionType
ALU = mybir.AluOpType

USE_FP8_SCORES = False


class Cfg:
    D = 1024      # model dim
    NHC = 8       # heads per core
    DH = 64       # head dim
    KL = 2048     # key length
    Q = 1024      # query length
    DI = 4096     # ffn inner
    LN_EPS = 1e-5
    N_CORES = 8

    HD = property(lambda s: s.NHC * s.DH)       # head dims per core (512)
    SCALE = property(lambda s: 1.0 / (s.DH ** 0.5))
    M = property(lambda s: s.KL - s.Q)          # mem length
    NS = property(lambda s: s.Q // 128)         # q tiles (8)
    DPT = property(lambda s: s.D // 128)        # 8
    HPT = property(lambda s: s.HD // 128)       # 4
    NTT = property(lambda s: s.KL // 128)       # 16
    BDW = property(lambda s: s.KL + 128)        # bd buffer width (2176)
    TOKF = property(lambda s: s.Q // 2)         # ffn tokens per core (512)

    def jm(self, s):
        """exact key extent for q-tile s: multiple of 128."""
        return 128 * (s + 9)


def _mm512(nc, ps, lhsT, rhs_fn, width, start, stop, lhsT_fn=None,
           perf_mode=None):
    """Issue width//512 matmuls of <=512 cols into ps[:, off:off+...]."""
    for nb in range(0, width, 512):
        ne = min(width, nb + 512)
        l = lhsT_fn(nb, ne) if lhsT_fn is not None else lhsT
        nc.tensor.matmul(ps[:, nb:ne], l, rhs_fn(nb, ne),
                         start=start, stop=stop, perf_mode=perf_mode)


def ts(i, n):
    return slice(i * n, (i + 1) * n)


def build_kernel(c: Cfg = None, collective=True, repeat=1):
    c = c or Cfg()
    nc = bacc.Bacc("TRN2", target_bir_lowering=False)

    io = {}
    def din(name, shape):
        io[name] = nc.dram_tensor(name, shape, F32, kind="ExternalInput")
    din("xt", [c.D, c.KL])        # pre-transposed concat(mems, w)
    din("rt", [c.D, c.KL])        # pre-transposed r
    din("qkvw", [c.D, 3 * c.HD])
    din("rnetw", [c.D, c.HD])
    din("oww", [c.HD, c.D])
    din("rwb", [1, c.HD])
    din("rrb", [1, c.HD])
    din("ln1g", [1, c.D]); din("ln1b", [1, c.D])
    din("ln2g", [1, c.D]); din("ln2b", [1, c.D])
    din("ffw1", [c.D, c.DI]); din("ffb1", [1, c.DI])
    din("ffw2", [c.DI, c.D]); din("ffb2", [1, c.D])
    din("wres", [c.TOKF, c.D])
    io["out"] = nc.dram_tensor("out", [c.TOKF, c.D], F32, kind="ExternalOutput")
    io["rs_bin"] = nc.dram_tensor("rs_bin", [c.Q, c.D], BF16)
    io["rs_bout"] = nc.dram_tensor("rs_bout", [c.TOKF, c.D], BF16)

    with tile.TileContext(nc) as tc:
        for _ in range(repeat):
            _body(tc, nc, c, io, collective=collective)
    nc.finalize()
    return nc


def _body(tc, nc, c, io, collective=True):
    ctx = contextlib.ExitStack()
    rg = [[i, i + 1] for i in range(0, c.N_CORES, 2)]
    with ctx:
        small = ctx.enter_context(tc.tile_pool(name="small", bufs=2))
        keep = ctx.enter_context(tc.tile_pool(name="keep", bufs=1))

        # manual pools, stack-disciplined: released in reverse alloc order,
        # with phR/wrp -> phX/wqp -> ptp pushed/popped in sequence so their
        # SBUF space is reused across phases.
        psA = tc.alloc_tile_pool(name="psA", bufs=2, space="PSUM")
        psD = tc.alloc_tile_pool(name="psD", bufs=2, space="PSUM")
        atp = tc.alloc_tile_pool(name="atp", bufs=1)
        owp = tc.alloc_tile_pool(name="owp", bufs=1)
        attk = tc.alloc_tile_pool(name="attk", bufs=1)
        stg = tc.alloc_tile_pool(name="stg", bufs=1)
        phR = tc.alloc_tile_pool(name="phR", bufs=1)
        wrp = tc.alloc_tile_pool(name="wrp", bufs=1)
        phX = tc.alloc_tile_pool(name="phX", bufs=1)
        wqp = tc.alloc_tile_pool(name="wqp", bufs=1)

        # ---- persistent small constants ----
        rwb_s = keep.tile([128, c.HPT], F32, tag="rwb")
        rrb_s = keep.tile([128, c.HPT], F32, tag="rrb")
        nc.sync.dma_start(out=rwb_s[:], in_=bass.AP(
            tensor=io["rwb"].ap().tensor, offset=0, ap=[[1, 128], [128, c.HPT]]))
        nc.sync.dma_start(out=rrb_s[:], in_=bass.AP(
            tensor=io["rrb"].ap().tensor, offset=0, ap=[[1, 128], [128, c.HPT]]))
        # fold the attention scale into the biases (applied at Q^T creation)
        nc.vector.tensor_scalar_mul(out=rwb_s[:], in0=rwb_s[:],
                                    scalar1=float(c.SCALE))
        nc.vector.tensor_scalar_mul(out=rrb_s[:], in0=rrb_s[:],
                                    scalar1=float(c.SCALE))


        # ============ phase 1: all input loads (gpsimd cast f32->bf16) =====
        rT = phR.tile([128, c.DPT * c.KL], BF16, tag="rT")
        xT = phX.tile([128, c.DPT * c.KL], BF16, tag="xT")
        wr_t, qkv_t = [], []
        for k in range(c.DPT):
            t = wrp.tile([128, c.HD], BF16, tag="wr_%d" % k)
            nc.gpsimd.dma_start(out=t[:], in_=io["rnetw"][ts(k, 128), :])
            wr_t.append(t)
            nc.gpsimd.dma_start(out=rT[:, ts(k, c.KL)],
                                in_=io["rt"][ts(k, 128), :])
        for k in range(c.DPT):
            t = wqp.tile([128, 3 * c.HD], BF16, tag="qkv_%d" % k)
            nc.gpsimd.dma_start(out=t[:], in_=io["qkvw"][ts(k, 128), :])
            qkv_t.append(t)
            nc.gpsimd.dma_start(out=xT[:, ts(k, c.KL)],
                                in_=io["xt"][ts(k, 128), :])

        def dr3(t, pair_w, w, off):
            """[K=128, 2, w] DoubleRow operand view of pair-tile t."""
            return bass.AP(tensor=t.tensor, offset=t.offset + off,
                           ap=[[2 * pair_w, 128], [pair_w, 2], [1, w]])

        # ============ phase 2: projections (fp8 DR, PSUM drains on DVE) ====
        VW = c.NHC * 65
        vb = attk.tile([128, c.NTT * VW], BF16, tag="vb")
        # fp8 split-layout score operands: one [64, 2*W] tile per head-pair;
        # head t2 at partition base 32*t2 with dh-halves adjacent in free dim.
        if USE_FP8_SCORES:
            kt8 = [attk.tile([64, 2 * c.KL], FP8, tag="kt8_%d" % i,
                             name="kt8_%d" % i) for i in range(c.HPT)]
            rtp8 = [attk.tile([64, 2 * c.KL], FP8, tag="rtp8_%d" % i,
                              name="rtp8_%d" % i) for i in range(c.HPT)]
            rwq8 = [attk.tile([64, 2 * c.Q], FP8, tag="rwq8_%d" % i,
                              name="rwq8_%d" % i) for i in range(c.HPT)]
            rrq8 = [attk.tile([64, 2 * c.Q], FP8, tag="rrq8_%d" % i,
                              name="rrq8_%d" % i) for i in range(c.HPT)]
        attnT = atp.tile([128, c.HPT * c.Q], BF16, tag="attnT")

        # bf16 staging for the projection outputs, released before attention.
        # rTp and kT share one tile (used sequentially around regroups).
        rTp = stg.tile([128, c.HPT * c.KL], BF16, tag="rTp")
        kT = rTp if USE_FP8_SCORES else stg.tile([128, c.HPT * c.KL], BF16,
                                                 tag="kT", name="kT")
        rwq = stg.tile([128, c.HPT * c.Q], BF16, tag="rwq")
        rrq = stg.tile([128, c.HPT * c.Q], BF16, tag="rrq")

        def regroup(dst_tiles, src, width):
            """bf16 [128, HPT*width] -> fp8 split layout (cast DMA)."""
            for hp in range(c.HPT):
                tl = dst_tiles[hp]
                for hh in range(2):
                    nc.gpsimd.dma_start(
                        out=bass.AP(tensor=tl.tensor,
                                    offset=tl.offset + hh * 32 * 2 * width,
                                    ap=[[2 * width, 32], [width, 2],
                                        [1, width]]),
                        in_=bass.AP(tensor=src.tensor,
                                    offset=src.offset + hp * width
                                    + hh * 64 * c.HPT * width,
                                    ap=[[c.HPT * width, 32],
                                        [32 * c.HPT * width, 2], [1, width]]))

        # rTp = (r @ r_net_w)^T  [hd-part, klen]; k-outer in two passes of
        # 4 psum chunks so the PE k-steps track the rT tile arrivals.
        def kouter_proj(dst, lhs_col_fn, rhs, out_cols):
            for half in range(2):
                chunks = [(m, ch) for m in (2 * half, 2 * half + 1)
                          for ch in range(2)]
                pss = {}
                for i, (m, ch) in enumerate(chunks):
                    pool = psA if i % 2 == 0 else psD
                    pss[(m, ch)] = pool.tile([128, 1024], F32,
                                             tag="a" if i % 2 == 0 else "d",
                                             name="ps_%d_%d" % (m, ch))
                for k in range(c.DPT):
                    for m, ch in chunks:
                        _mm512(nc, pss[(m, ch)], lhs_col_fn(k, m),
                               lambda nb, ne, k=k, ch=ch:
                                   rhs[:, k * c.KL + ch * 1024 + nb:
                                       k * c.KL + ch * 1024 + ne],
                               1024, start=(k == 0), stop=(k == c.DPT - 1))
                for m, ch in chunks:
                    nc.vector.tensor_copy(
                        out=dst[:, m * out_cols + ch * 1024:
                                m * out_cols + (ch + 1) * 1024],
                        in_=pss[(m, ch)][:])

        kouter_proj(rTp, lambda k, m: wr_t[k][:, ts(m, 128)], rT, c.KL)
        if USE_FP8_SCORES:
            regroup(rtp8, rTp, c.KL)


        # K^T [hd-part, klen]
        kouter_proj(kT, lambda k, m: qkv_t[k][:, c.HD + m * 128:
                                              c.HD + (m + 1) * 128], xT, c.KL)
        if USE_FP8_SCORES:
            regroup(kt8, kT, c.KL)
        # V natural [klen-part, hd] (+ ones col per head for softmax denom)
        for jt in range(c.NTT):
            ps = psD.tile([128, 1024], F32, tag="d")
            for k in range(c.DPT):
                nc.tensor.matmul(
                    ps[:, 0:c.HD],
                    xT[:, k * c.KL + jt * 128: k * c.KL + (jt + 1) * 128],
                    qkv_t[k][:, 2 * c.HD: 3 * c.HD],
                    start=(k == 0), stop=(k == c.DPT - 1))
            dst = bass.AP(
                tensor=vb.tensor, offset=vb.offset + jt * VW,
                ap=[[c.NTT * VW, 128], [65, c.NHC], [1, c.DH]])
            nc.vector.tensor_copy(out=dst, in_=ps[:, 0:c.HD])
            ones = bass.AP(
                tensor=vb.tensor, offset=vb.offset + jt * VW + c.DH,
                ap=[[c.NTT * VW, 128], [65, c.NHC], [1, 1]])
            nc.vector.memset(ones, 1.0)
        # Q^T with scale and biases folded: rwq = SCALE*q + SCALE*rwb etc.
        for m in range(c.HPT):
            ps = psA.tile([128, 1024], F32, tag="a")
            for k in range(c.DPT):
                _mm512(nc, ps, qkv_t[k][:, ts(m, 128)],
                       lambda nb, ne, k=k: xT[:, k * c.KL + c.M + nb:
                                              k * c.KL + c.M + ne],
                       1024, start=(k == 0), stop=(k == c.DPT - 1))
            sl = ts(m, c.Q)
            nc.scalar.activation(out=rwq[:, sl], in_=ps[:],
                                 func=AF.Identity, bias=rwb_s[:, m:m + 1],
                                 scale=float(c.SCALE))
            nc.vector.tensor_scalar(out=rrq[:, sl], in0=ps[:],
                                    scalar1=rrb_s[:, m:m + 1],
                                    scalar2=float(c.SCALE),
                                    op0=ALU.mult, op1=ALU.add)
        if USE_FP8_SCORES:
            regroup(rwq8, rwq, c.Q)
            regroup(rrq8, rrq, c.Q)
        wqp.release()
        phX.release()
        wrp.release()
        phR.release()
        if USE_FP8_SCORES:
            stg.release()
        # ---- score-pipeline rings (allocated in the freed load space) ----
        # bdw: exp(BD) in absolute r-coords [0, 2048) + permanent zero tail
        # [2048, 2176) which realizes the causal mask (exp(-inf) = 0).
        sbp = tc.alloc_tile_pool(name="sbp", bufs=1)
        bdws, sbAs, sbBs = [], [], []
        for i in range(2):
            b = sbp.tile([128, 2 * c.BDW], BF16, tag="bdw%d" % i, name="bdw%d" % i)
            nc.vector.memset(bass.AP(
                tensor=b.tensor, offset=b.offset + c.KL,
                ap=[[2 * c.BDW, 128], [c.BDW, 2], [1, 128]]), 0.0)
            bdws.append(b)
            sbBs.append(sbp.tile([128, 2 * c.KL], BF16, tag="sbB%d" % i, name="sbB%d" % i))
            sbAs.append(sbp.tile([128, 2 * c.KL], BF16, tag="sbA%d" % i, name="sbA%d" % i))

        # ============ phase 3: attention (exp-split) ============
        ptp = tc.alloc_tile_pool(name="ptp", bufs=2)
        ow_t = []
        for p in range(c.HPT):
            t = owp.tile([128, c.D], BF16, tag="ow_%d" % p)
            nc.gpsimd.dma_start(out=t[:], in_=io["oww"][ts(p, 128), :])
            ow_t.append(t)
        pend = []

        def score_mm(out_ap, q8, k8, hp, t2, s, koff, w):
            """fp8 DoubleRow score matmul: contraction 64 as [32, 2, .]."""
            tl_q, tl_k = q8[hp], k8[hp]
            hb = 32 * t2
            nc.tensor.matmul(
                out_ap,
                bass.AP(tensor=tl_q.tensor,
                        offset=tl_q.offset + hb * 2 * c.Q + s * 128,
                        ap=[[2 * c.Q, 32], [c.Q, 2], [1, 128]]),
                bass.AP(tensor=tl_k.tensor,
                        offset=tl_k.offset + hb * 2 * c.KL + koff,
                        ap=[[2 * c.KL, 32], [c.KL, 2], [1, w]]),
                start=True, stop=True, perf_mode=DR)

        def score_mm_bf16(out_ap, qst, kst, hp, t2, s, koff, w):
            hr = t2 * 64
            nc.tensor.matmul(
                out_ap,
                qst[hr:hr + 64, hp * c.Q + s * 128: hp * c.Q + (s + 1) * 128],
                kst[hr:hr + 64, hp * c.KL + koff: hp * c.KL + koff + w],
                start=True, stop=True)

        def score_iter(hp, s, pT_A, pT_B):
            """BD + AC pair-chunks with immediate Act exp drains."""
            jmx = c.jm(s)
            wst = c.Q - 128 * (s + 1)
            slot = (hp * c.NS + s) % 2
            bdwt, sba = bdws[slot], sbAs[slot]
            for lo in range(0, jmx, 512):
                w = min(512, jmx - lo)
                ps = psD.tile([128, 1024], F32, tag="d")
                for t2 in range(2):
                    if USE_FP8_SCORES:
                        score_mm(ps[:, t2 * 512: t2 * 512 + w], rrq8, rtp8,
                                 hp, t2, s, wst + lo, w)
                    else:
                        score_mm_bf16(ps[:, t2 * 512: t2 * 512 + w], rrq, rTp,
                                      hp, t2, s, wst + lo, w)
                nc.scalar.activation(
                    out=bass.AP(tensor=bdwt.tensor,
                                offset=bdwt.offset + wst + lo,
                                ap=[[2 * c.BDW, 128], [c.BDW, 2], [1, w]]),
                    in_=bass.AP(tensor=ps.tensor, offset=ps.offset,
                                ap=[[1024, 128], [512, 2], [1, w]]),
                    func=AF.Exp)
            for lo in range(0, jmx, 512):
                w = min(512, jmx - lo)
                ps = psA.tile([128, 1024], F32, tag="a")
                for t2 in range(2):
                    if USE_FP8_SCORES:
                        score_mm(ps[:, t2 * 512: t2 * 512 + w], rwq8, kt8,
                                 hp, t2, s, lo, w)
                    else:
                        score_mm_bf16(ps[:, t2 * 512: t2 * 512 + w], rwq, kT,
                                      hp, t2, s, lo, w)
                nc.scalar.activation(
                    out=bass.AP(tensor=sba.tensor, offset=sba.offset + lo,
                                ap=[[2 * c.KL, 128], [c.KL, 2], [1, w]]),
                    in_=bass.AP(tensor=ps.tensor, offset=ps.offset,
                                ap=[[1024, 128], [512, 2], [1, w]]),
                    func=AF.Exp)
            pend.append((s, slot))

        pend_tp = []

        def flush_dm():
            """rel-shift diag copy + exp-product for the oldest pending s."""
            s, slot = pend.pop(0)
            jmx = c.jm(s)
            wst = c.Q - 128 * (s + 1)
            bdwt, sba, sbb = bdws[slot], sbAs[slot], sbBs[slot]
            # rel-shift: sbB[p, h2, j] = exp(BD)[p, h2, wst + 127 - p + j]
            nc.gpsimd.dma_start(
                out=bass.AP(tensor=sbb.tensor, offset=sbb.offset,
                            ap=[[2 * c.KL, 128], [c.KL, 2], [1, jmx]]),
                in_=bass.AP(tensor=bdwt.tensor,
                            offset=bdwt.offset + wst + 127,
                            ap=[[2 * c.BDW - 1, 128], [c.BDW, 2], [1, jmx]]))
            # p = exp(AC) * exp(BD)_shifted  (DVE 2x, in place into sbB)
            b3 = bass.AP(tensor=sbb.tensor, offset=sbb.offset,
                         ap=[[2 * c.KL, 128], [c.KL, 2], [1, jmx]])
            nc.vector.tensor_tensor(
                out=b3,
                in0=bass.AP(tensor=sba.tensor, offset=sba.offset,
                            ap=[[2 * c.KL, 128], [c.KL, 2], [1, jmx]]),
                in1=b3, op=ALU.mult)
            pend_tp.append((s, slot))

        def flush_tp(pT_A, pT_B):
            s, slot = pend_tp.pop(0)
            jmx = c.jm(s)
            sbb = sbBs[slot]
            for t2, pT in ((0, pT_A), (1, pT_B)):
                nc.sync.dma_start(
                    out=bass.AP(tensor=pT.tensor, offset=pT.offset + s * 128,
                                ap=[[c.NTT * c.Q, 128], [c.Q, jmx // 128], [1, 128]]),
                    in_=bass.AP(tensor=sbb.tensor,
                                offset=sbb.offset + t2 * c.KL,
                                ap=[[2 * c.KL, 128], [1, jmx]]),
                    transpose=True)

        def issue_pv(hp, t2, pT):
            h = 2 * hp + t2
            hr = t2 * 64
            ps = psA.tile([128, 1024], F32, tag="a")
            for c2 in range(2):
                lo, hi = c2 * 512, (c2 + 1) * 512
                njt = 12 if c2 == 0 else 16
                out = ps[0:65, c2 * 512:(c2 + 1) * 512]
                for jt in range(njt):
                    nlo = max(lo, 128 * (jt - 8))
                    nc.tensor.matmul(
                        out[:, nlo - lo:512],
                        vb[:, jt * VW + h * 65: jt * VW + h * 65 + 65],
                        pT[:, jt * c.Q + nlo: jt * c.Q + hi],
                        start=(jt == 0), stop=(jt == njt - 1))
            for c2 in range(2):
                sl = slice(c2 * 512, (c2 + 1) * 512)
                rd = small.tile([1, 512], F32, tag="rd")
                nc.vector.reciprocal(out=rd[:], in_=ps[64:65, sl])
                rdb = small.tile([64, 512], F32, tag="rdb")
                nc.gpsimd.partition_broadcast(rdb[:], rd[:])
                nc.vector.tensor_tensor(
                    out=attnT[hr:hr + 64, hp * c.Q + c2 * 512:
                              hp * c.Q + (c2 + 1) * 512],
                    in0=ps[0:64, sl], in1=rdb[:], op=ALU.mult)

        prev = None
        for hp in range(c.NHC // 2):
            pT_A = pT_B = None
            for s in range(c.NS):
                score_iter(hp, s, None, None)
                if prev is not None and s in (2, 3):
                    issue_pv(prev[0], s - 2, prev[s - 1])
                if s == 3:
                    pT_A = ptp.tile([128, c.NTT * c.Q], BF16, tag="pT",
                                    name="pTA")
                    pT_B = ptp.tile([128, c.NTT * c.Q], BF16, tag="pT",
                                    name="pTB")
                # transposes (readers of sbB slot s-3) must precede the
                # diag copy of s-1 (writer of the same slot)
                if len(pend_tp) > 1:
                    flush_tp(pT_A, pT_B)
                if len(pend) > 1:
                    flush_dm()
            flush_tp(pT_A, pT_B)   # tp(4)
            flush_tp(pT_A, pT_B)   # tp(5)
            flush_dm()             # dm(7)
            while pend_tp:
                flush_tp(pT_A, pT_B)
            prev = (hp, pT_A, pT_B)
        issue_pv(prev[0], 0, prev[1])
        issue_pv(prev[0], 1, prev[2])

        ptp.release()
        sbp.release()
        if not USE_FP8_SCORES:
            stg.release()
        attk.release()

        # ============ phase 4: o_proj (natural out) -> ReduceScatter ============
        with tc.tile_pool(name="stO", bufs=3) as stage:
            for qb in range(c.NS):
                ost = stage.tile([128, c.D], BF16, tag="ost")
                for half in range(2):
                    ps = psD.tile([128, 1024], F32, tag="d")
                    for k in range(c.HPT):
                        nc.tensor.matmul(
                            ps[:, 0:512],
                            attnT[:, k * c.Q + qb * 128: k * c.Q + (qb + 1) * 128],
                            ow_t[k][:, half * 512: (half + 1) * 512],
                            start=(k == 0), stop=(k == c.HPT - 1))
                    nc.scalar.activation(out=ost[:, half * 512:(half + 1) * 512],
                                         in_=ps[:, 0:512], func=AF.Copy)
                nc.sync.dma_start(out=io["rs_bin"][ts(qb, 128), :], in_=ost[:])
        owp.release()
        atp.release()
        psD.release()
        psA.release()

        # ============ phase 5: LN1 + FFN + LN2 ============
        w1p = ctx.enter_context(tc.tile_pool(name="w1p", bufs=1))
        w1_t = []
        for k in range(c.DPT):
            t = w1p.tile([128, c.DI], BF16, tag="w1_%d" % k)
            nc.gpsimd.dma_start(out=t[:], in_=io["ffw1"][ts(k, 128), :])
            w1_t.append(t)

        phE = ctx.enter_context(tc.tile_pool(name="phE", bufs=1))
        eps_t = phE.tile([128, 1], F32, tag="eps")
        nc.vector.memset(eps_t[:], c.LN_EPS)
        lns = {}
        for nm in ("ln1g", "ln1b", "ln2g", "ln2b"):
            tl = phE.tile([128, c.D], F32, tag=nm)
            bcast = bass.AP(tensor=io[nm].ap().tensor, offset=0,
                            ap=[[0, 128], [1, c.D]])
            nc.sync.dma_start(out=tl[:], in_=bcast)
            lns[nm] = tl
        fb1 = phE.tile([128, c.DI // 128], F32, tag="fb1")
        nc.sync.dma_start(out=fb1[:], in_=bass.AP(
            tensor=io["ffb1"].ap().tensor, offset=0, ap=[[1, 128], [128, c.DI // 128]]))
        fb2n = phE.tile([128, c.D], F32, tag="fb2n")
        nc.sync.dma_start(out=fb2n[:], in_=bass.AP(
            tensor=io["ffb2"].ap().tensor, offset=0, ap=[[0, 128], [1, c.D]]))

        ntt = c.TOKF // 128  # 4
        ffn = ctx.enter_context(tc.tile_pool(name="ffn", bufs=1))
        ln1r = ffn.tile([128, ntt * c.D], BF16, tag="ln1r")  # ln1 out + b2
        lnT = ffn.tile([128, c.DPT * c.TOKF], BF16, tag="lnT")
        hT = ffn.tile([128, (c.DI // 128) * c.TOKF], BF16, tag="hT")
        wres4 = ffn.tile([128, ntt * c.D], BF16, tag="wres4")
        for tt in range(ntt):
            nc.gpsimd.dma_start(out=wres4[:, ts(tt, c.D)],
                                in_=io["wres"][ts(tt, 128), :])

        if collective:
            nc.gpsimd.collective_compute(
                "ReduceScatter", ALU.add, replica_groups=rg,
                ins=[io["rs_bin"].ap().opt()], outs=[io["rs_bout"].ap().opt()])
        else:
            nc.sync.dma_start(out=io["rs_bout"].ap().opt(),
                              in_=io["rs_bin"].ap()[0:c.TOKF, :].opt())

        # 8 single-bank accumulators for the k-outer FFN2 (and FFN1/LN use)
        psF = ctx.enter_context(tc.tile_pool(name="psF", bufs=1, space="PSUM"))

        def psf(i, shape, dtype=F32):
            return psF.tile(shape, dtype, tag="p%d" % (i % 8),
                            name="psf%d" % (i % 8))

        with tc.tile_pool(name="stE", bufs=2) as stage, \
             tc.tile_pool(name="w2s", bufs=8) as w2s:
            for tt in range(ntt):
                zb = stage.tile([128, c.D], BF16, tag="zb")
                nc.sync.dma_start(out=zb[:], in_=io["rs_bout"][ts(tt, 128), :])
                z = stage.tile([128, c.D], F32, tag="z")
                nc.vector.tensor_tensor(out=z[:], in0=wres4[:, ts(tt, c.D)],
                                        in1=zb[:], op=ALU.add)
                lsl = slice(tt * c.D, (tt + 1) * c.D)
                _layernorm_nat(nc, c, small, z[:], eps_t,
                               lns["ln1g"], lns["ln1b"], ln1r[:, lsl])
                znb = stage.tile([128, c.D], BF16, tag="znb")
                nc.vector.tensor_copy(out=znb[:], in_=ln1r[:, lsl])
                dstap = bass.AP(
                    tensor=lnT.tensor, offset=lnT.offset + tt * 128,
                    ap=[[c.DPT * c.TOKF, 128], [c.TOKF, c.DPT], [1, 128]])
                nc.sync.dma_start(out=dstap, in_=znb[:], transpose=True)
                # pre-add b2 for the FFN2 residual
                nc.vector.tensor_tensor(out=ln1r[:, lsl], in0=ln1r[:, lsl],
                                        in1=fb2n[:], op=ALU.add)
            # FFN1: hT[di, tok], m-outer with resident w1; token-halves so
            # the first half starts after only 2 of 4 LN1 tiles
            for g in range(2):
                for m in range(c.DI // 128):
                    ps = psf(m, [128, 256])
                    for k in range(c.DPT):
                        nc.tensor.matmul(
                            ps[:], w1_t[k][:, ts(m, 128)],
                            lnT[:, k * c.TOKF + g * 256: k * c.TOKF + (g + 1) * 256],
                            start=(k == 0), stop=(k == c.DPT - 1))
                    nc.scalar.activation(
                        out=hT[:, m * c.TOKF + g * 256: m * c.TOKF + (g + 1) * 256],
                        in_=ps[:], func=AF.Relu, bias=fb1[:, m:m + 1])
            # FFN2: k-outer, streaming w2, natural out [tok, d]
            nkt = c.DI // 128
            acc = [psf(i, [128, 512]) for i in range(8)]
            for k in range(nkt):
                w2t = w2s.tile([128, c.D], BF16, tag="w2")
                nc.gpsimd.dma_start(out=w2t[:], in_=io["ffw2"][ts(k, 128), :])
                for tb in range(ntt):
                    for half in range(2):
                        nc.tensor.matmul(
                            acc[tb * 2 + half][:],
                            hT[:, k * c.TOKF + tb * 128: k * c.TOKF + (tb + 1) * 128],
                            w2t[:, half * 512:(half + 1) * 512],
                            start=(k == 0), stop=(k == nkt - 1))
            for tb in range(ntt):
                o2n = stage.tile([128, c.D], F32, tag="o2n")
                for half in range(2):
                    nc.vector.tensor_tensor(
                        out=o2n[:, half * 512:(half + 1) * 512],
                        in0=acc[tb * 2 + half][:],
                        in1=ln1r[:, tb * c.D + half * 512: tb * c.D + (half + 1) * 512],
                        op=ALU.add)
                fin = stage.tile([128, c.D], F32, tag="fin")
                _layernorm_nat(nc, c, small, o2n[:], eps_t,
                               lns["ln2g"], lns["ln2b"], fin[:])
                nc.sync.dma_start(out=io["out"][ts(tb, 128), :], in_=fin[:])


def _layernorm_nat(nc, c, small, z, eps_t, g, b, out_dst):
    """LayerNorm over the free axis of z [128, D] fp32."""
    BN_FMAX = nc.vector.BN_STATS_FMAX
    d = z.shape[-1]
    sub = math.gcd(BN_FMAX, d)
    nsub = d // sub
    zr = z.rearrange("p (n f) -> p n f", f=sub)
    stats = small.tile([128, nsub, nc.vector.BN_STATS_DIM], F32, tag="bnst")
    for i in range(nsub):
        nc.vector.bn_stats(out=stats[:, i, :], in_=zr[:, i, :])
    mv = small.tile([128, nc.vector.BN_AGGR_DIM], F32, tag="bnag")
    nc.vector.bn_aggr(out=mv[:], in_=stats[:])
    mean, var = mv[:, 0:1], mv[:, 1:2]
    nc.scalar.activation(out=var, in_=var, func=AF.Sqrt, bias=eps_t[:], scale=1.0)
    nc.vector.reciprocal(out=var, in_=var)
    nc.vector.tensor_scalar(out=out_dst, in0=z, scalar1=mean, scalar2=var,
                            op0=ALU.subtract, op1=ALU.mult)
    nc.vector.tensor_tensor(out=out_dst, in0=out_dst, in1=g[:, 0:d], op=ALU.mult)
    nc.vector.tensor_tensor(out=out_dst, in0=out_dst, in1=b[:, 0:d], op=ALU.add)


# ============================================================
# host-side sharding + entry point
# ============================================================

def shard_inputs(inputs, c: Cfg = None):
    c = c or Cfg()
    w = np.asarray(inputs["w"], np.float32)
    r = np.asarray(inputs["r"], np.float32)
    mems = np.asarray(inputs["mems"], np.float32)
    qkv_w = np.asarray(inputs["qkv_w"], np.float32)
    r_net_w = np.asarray(inputs["r_net_w"], np.float32)
    o_w = np.asarray(inputs["o_w"], np.float32)
    r_w_bias = np.asarray(inputs["r_w_bias"], np.float32).reshape(-1)
    r_r_bias = np.asarray(inputs["r_r_bias"], np.float32).reshape(-1)
    NHD = qkv_w.shape[1] // 3
    rt = np.ascontiguousarray(r[:, 0, :].T)
    in_maps = []
    for core in range(c.N_CORES):
        b, hh = core // 2, core % 2
        hsl = slice(hh * c.HD, (hh + 1) * c.HD)
        xt_c = np.concatenate([mems[:, b, :], w[:, b, :]], axis=0).T
        qkvw_c = np.concatenate([qkv_w[:, j * NHD + hh * c.HD:
                                       j * NHD + (hh + 1) * c.HD]
                                 for j in range(3)], axis=1)
        in_maps.append({
            "xt": np.ascontiguousarray(xt_c),
            "rt": rt,
            "qkvw": np.ascontiguousarray(qkvw_c),
            "rnetw": np.ascontiguousarray(r_net_w[:, hsl]),
            "oww": np.ascontiguousarray(o_w[hsl, :]),
            "rwb": np.ascontiguousarray(r_w_bias[hsl][None, :]),
            "rrb": np.ascontiguousarray(r_r_bias[hsl][None, :]),
            "ln1g": np.asarray(inputs["ln1_g"], np.float32).reshape(1, -1),
            "ln1b": np.asarray(inputs["ln1_b"], np.float32).reshape(1, -1),
            "ln2g": np.asarray(inputs["ln2_g"], np.float32).reshape(1, -1),
            "ln2b": np.asarray(inputs["ln2_b"], np.float32).reshape(1, -1),
            "ffw1": np.asarray(inputs["ff_w1"], np.float32),
            "ffb1": np.asarray(inputs["ff_b1"], np.float32).reshape(1, -1),
            "ffw2": np.asarray(inputs["ff_w2"], np.float32),
            "ffb2": np.asarray(inputs["ff_b2"], np.float32).reshape(1, -1),
            "wres": np.ascontiguousarray(w[hh * c.TOKF:(hh + 1) * c.TOKF, b, :]),
        })
    return in_maps


def unshard_output(results, inputs, c: Cfg = None):
    c = c or Cfg()
    w = np.asarray(inputs["w"])
    Q, B, D = w.shape
    out = np.zeros((Q, B, D), np.float32)
    for core in range(c.N_CORES):
        b, hh = core // 2, core % 2
        out[hh * c.TOKF:(hh + 1) * c.TOKF, b, :] = results[core]["out"]
    return out


_NC_CACHE = {}


def kernel(**inputs):
    if "nc" not in _NC_CACHE:
        _NC_CACHE["nc"] = build_kernel()
    nc = _NC_CACHE["nc"]
    in_maps = shard_inputs(inputs)
    from concourse.bass_utils import run_bass_kernel_spmd
    res = run_bass_kernel_spmd(nc, in_maps, core_ids=list(range(Cfg.N_CORES)))
    return unshard_output(res.results, inputs)


# revision 47
# speedup vs baseline: 1.0045x; 1.0045x over previous
"""Trainium2 Bass kernel for nn_MemTransformerLM (Transformer-XL layer).

Sharding (8 cores): batch (4) x head-half (2). Core c handles batch b = c//2
and heads [hh*8, hh*8+8), hh = c%2, for all 1024 queries. After o_proj a
2-rank ReduceScatter over core pairs (2b, 2b+1) splits tokens for the FFN:
even core keeps tokens [0,512), odd [512,1024).

v2 rewrite ("exp-split" attention):
 - host pre-transposes x/r -> direct [d-part, klen] loads, no on-chip
   input transposes.
 - softmax split: p = exp(AC) * exp(BD_shifted). Act exponentiates the
   AC and BD PSUM chunks directly into SBUF (no PSUM-drain copies), a
   single gpsimd DMA applies the Transformer-XL rel-shift as a diagonal
   copy, and DVE multiplies the two factors at 2x (all-bf16).
 - the causal/memory mask is a permanently-zeroed tail of the BD buffer
   (exp(-inf) = 0), so masking costs nothing per iteration.
 - PV softmax denominator via a ones-column in V; reciprocal broadcast
   uses gpsimd partition_broadcast instead of a DMA.
 - PV of pair hp-1 is issued at s=2/3 of pair hp (after the previous
   pair's transpose tail has landed) to keep PE fed without stalls.
"""

import contextlib
import math

import numpy as np

import concourse.bass as bass
import concourse.bacc as bacc
import concourse.mybir as mybir
import concourse.tile as tile

F32 = mybir.dt.float32
BF16 = mybir.dt.bfloat16
FP8 = mybir.dt.float8e4
DR = mybir.MatmulPerfMode.DoubleRow
AF = mybir.ActivationFunctionType
ALU = mybir.AluOpType

USE_FP8_SCORES = False


class Cfg:
    D = 1024      # model dim
    NHC = 8       # heads per core
    DH = 64       # head dim
    KL = 2048     # key length
    Q = 1024      # query length
    DI = 4096     # ffn inner
    LN_EPS = 1e-5
    N_CORES = 8

    HD = property(lambda s: s.NHC * s.DH)       # head dims per core (512)
    SCALE = property(lambda s: 1.0 / (s.DH ** 0.5))
    M = property(lambda s: s.KL - s.Q)          # mem length
    NS = property(lambda s: s.Q // 128)         # q tiles (8)
    DPT = property(lambda s: s.D // 128)        # 8
    HPT = property(lambda s: s.HD // 128)       # 4
    NTT = property(lambda s: s.KL // 128)       # 16
    BDW = property(lambda s: s.KL + 128)        # bd buffer width (2176)
    TOKF = property(lambda s: s.Q // 2)         # ffn tokens per core (512)

    def jm(self, s):
        """exact key extent for q-tile s: multiple of 128."""
        return 128 * (s + 9)


def _mm512(nc, ps, lhsT, rhs_fn, width, start, stop, lhsT_fn=None,
           perf_mode=None):
    """Issue width//512 matmuls of <=512 cols into ps[:, off:off+...]."""
    for nb in range(0, width, 512):
        ne = min(width, nb + 512)
        l = lhsT_fn(nb, ne) if lhsT_fn is not None else lhsT
        nc.tensor.matmul(ps[:, nb:ne], l, rhs_fn(nb, ne),
                         start=start, stop=stop, perf_mode=perf_mode)


def ts(i, n):
    return slice(i * n, (i + 1) * n)


def build_kernel(c: Cfg = None, collective=True, repeat=1):
    c = c or Cfg()
    nc = bacc.Bacc("TRN2", target_bir_lowering=False)

    io = {}
    def din(name, shape):
        io[name] = nc.dram_tensor(name, shape, F32, kind="ExternalInput")
    din("xt", [c.D, c.KL])        # pre-transposed concat(mems, w)
    din("rt", [c.D, c.KL])        # pre-transposed r
    din("qkvw", [c.D, 3 * c.HD])
    din("rnetw", [c.D, c.HD])
    din("oww", [c.HD, c.D])
    din("rwb", [1, c.HD])
    din("rrb", [1, c.HD])
    din("ln1g", [1, c.D]); din("ln1b", [1, c.D])
    din("ln2g", [1, c.D]); din("ln2b", [1, c.D])
    din("ffw1", [c.D, c.DI]); din("ffb1", [1, c.DI])
    din("ffw2", [c.DI, c.D]); din("ffb2", [1, c.D])
    din("wres", [c.TOKF, c.D])
    io["out"] = nc.dram_tensor("out", [c.TOKF, c.D], F32, kind="ExternalOutput")
    io["rs_bin"] = nc.dram_tensor("rs_bin", [c.Q, c.D], BF16)
    io["rs_bout"] = nc.dram_tensor("rs_bout", [c.TOKF, c.D], BF16)

    with tile.TileContext(nc) as tc:
        for _ in range(repeat):
            _body(tc, nc, c, io, collective=collective)
    nc.finalize()
    return nc


def _body(tc, nc, c, io, collective=True):
    ctx = contextlib.ExitStack()
    rg = [[i, i + 1] for i in range(0, c.N_CORES, 2)]
    with ctx:
        small = ctx.enter_context(tc.tile_pool(name="small", bufs=2))
        keep = ctx.enter_context(tc.tile_pool(name="keep", bufs=1))

        # manual pools, stack-disciplined: released in reverse alloc order,
        # with phR/wrp -> phX/wqp -> ptp pushed/popped in sequence so their
        # SBUF space is reused across phases.
        psA = tc.alloc_tile_pool(name="psA", bufs=2, space="PSUM")
        psD = tc.alloc_tile_pool(name="psD", bufs=2, space="PSUM")
        atp = tc.alloc_tile_pool(name="atp", bufs=1)
        owp = tc.alloc_tile_pool(name="owp", bufs=1)
        attk = tc.alloc_tile_pool(name="attk", bufs=1)
        stg = tc.alloc_tile_pool(name="stg", bufs=1)
        phR = tc.alloc_tile_pool(name="phR", bufs=1)
        wrp = tc.alloc_tile_pool(name="wrp", bufs=1)
        phX = tc.alloc_tile_pool(name="phX", bufs=1)
        wqp = tc.alloc_tile_pool(name="wqp", bufs=1)

        # ---- persistent small constants ----
        rwb_s = keep.tile([128, c.HPT], F32, tag="rwb")
        rrb_s = keep.tile([128, c.HPT], F32, tag="rrb")
        nc.sync.dma_start(out=rwb_s[:], in_=bass.AP(
            tensor=io["rwb"].ap().tensor, offset=0, ap=[[1, 128], [128, c.HPT]]))
        nc.sync.dma_start(out=rrb_s[:], in_=bass.AP(
            tensor=io["rrb"].ap().tensor, offset=0, ap=[[1, 128], [128, c.HPT]]))
        # fold the attention scale into the biases (applied at Q^T creation)
        nc.vector.tensor_scalar_mul(out=rwb_s[:], in0=rwb_s[:],
                                    scalar1=float(c.SCALE))
        nc.vector.tensor_scalar_mul(out=rrb_s[:], in0=rrb_s[:],
                                    scalar1=float(c.SCALE))


        # ============ phase 1: all input loads (gpsimd cast f32->bf16) =====
        rT = phR.tile([128, c.DPT * c.KL], BF16, tag="rT")
        xT = phX.tile([128, c.DPT * c.KL], BF16, tag="xT")
        wr_t, qkv_t = [], []
        for k in range(c.DPT):
            t = wrp.tile([128, c.HD], BF16, tag="wr_%d" % k)
            nc.gpsimd.dma_start(out=t[:], in_=io["rnetw"][ts(k, 128), :])
            wr_t.append(t)
            nc.gpsimd.dma_start(out=rT[:, ts(k, c.KL)],
                                in_=io["rt"][ts(k, 128), :])
        for k in range(c.DPT):
            t = wqp.tile([128, 3 * c.HD], BF16, tag="qkv_%d" % k)
            nc.gpsimd.dma_start(out=t[:], in_=io["qkvw"][ts(k, 128), :])
            qkv_t.append(t)
            nc.gpsimd.dma_start(out=xT[:, ts(k, c.KL)],
                                in_=io["xt"][ts(k, 128), :])

        def dr3(t, pair_w, w, off):
            """[K=128, 2, w] DoubleRow operand view of pair-tile t."""
            return bass.AP(tensor=t.tensor, offset=t.offset + off,
                           ap=[[2 * pair_w, 128], [pair_w, 2], [1, w]])

        # ============ phase 2: projections (fp8 DR, PSUM drains on DVE) ====
        VW = c.NHC * 65
        vb = attk.tile([128, c.NTT * VW], BF16, tag="vb")
        # fp8 split-layout score operands: one [64, 2*W] tile per head-pair;
        # head t2 at partition base 32*t2 with dh-halves adjacent in free dim.
        if USE_FP8_SCORES:
            kt8 = [attk.tile([64, 2 * c.KL], FP8, tag="kt8_%d" % i,
                             name="kt8_%d" % i) for i in range(c.HPT)]
            rtp8 = [attk.tile([64, 2 * c.KL], FP8, tag="rtp8_%d" % i,
                              name="rtp8_%d" % i) for i in range(c.HPT)]
            rwq8 = [attk.tile([64, 2 * c.Q], FP8, tag="rwq8_%d" % i,
                              name="rwq8_%d" % i) for i in range(c.HPT)]
            rrq8 = [attk.tile([64, 2 * c.Q], FP8, tag="rrq8_%d" % i,
                              name="rrq8_%d" % i) for i in range(c.HPT)]
        attnT = atp.tile([128, c.HPT * c.Q], BF16, tag="attnT")

        # bf16 staging for the projection outputs, released before attention.
        # rTp and kT share one tile (used sequentially around regroups).
        rTp = stg.tile([128, c.HPT * c.KL], BF16, tag="rTp")
        kT = rTp if USE_FP8_SCORES else stg.tile([128, c.HPT * c.KL], BF16,
                                                 tag="kT", name="kT")
        rwq = stg.tile([128, c.HPT * c.Q], BF16, tag="rwq")
        rrq = stg.tile([128, c.HPT * c.Q], BF16, tag="rrq")

        def regroup(dst_tiles, src, width):
            """bf16 [128, HPT*width] -> fp8 split layout (cast DMA)."""
            for hp in range(c.HPT):
                tl = dst_tiles[hp]
                for hh in range(2):
                    nc.gpsimd.dma_start(
                        out=bass.AP(tensor=tl.tensor,
                                    offset=tl.offset + hh * 32 * 2 * width,
                                    ap=[[2 * width, 32], [width, 2],
                                        [1, width]]),
                        in_=bass.AP(tensor=src.tensor,
                                    offset=src.offset + hp * width
                                    + hh * 64 * c.HPT * width,
                                    ap=[[c.HPT * width, 32],
                                        [32 * c.HPT * width, 2], [1, width]]))

        # rTp = (r @ r_net_w)^T  [hd-part, klen]; k-outer in two passes of
        # 4 psum chunks so the PE k-steps track the rT tile arrivals.
        def kouter_proj(dst, lhs_col_fn, rhs, out_cols):
            for half in range(2):
                chunks = [(m, ch) for m in (2 * half, 2 * half + 1)
                          for ch in range(2)]
                pss = {}
                for i, (m, ch) in enumerate(chunks):
                    pool = psA if i % 2 == 0 else psD
                    pss[(m, ch)] = pool.tile([128, 1024], F32,
                                             tag="a" if i % 2 == 0 else "d",
                                             name="ps_%d_%d" % (m, ch))
                for k in range(c.DPT):
                    for m, ch in chunks:
                        _mm512(nc, pss[(m, ch)], lhs_col_fn(k, m),
                               lambda nb, ne, k=k, ch=ch:
                                   rhs[:, k * c.KL + ch * 1024 + nb:
                                       k * c.KL + ch * 1024 + ne],
                               1024, start=(k == 0), stop=(k == c.DPT - 1))
                for m, ch in chunks:
                    nc.vector.tensor_copy(
                        out=dst[:, m * out_cols + ch * 1024:
                                m * out_cols + (ch + 1) * 1024],
                        in_=pss[(m, ch)][:])

        kouter_proj(rTp, lambda k, m: wr_t[k][:, ts(m, 128)], rT, c.KL)
        if USE_FP8_SCORES:
            regroup(rtp8, rTp, c.KL)


        # K^T [hd-part, klen]
        kouter_proj(kT, lambda k, m: qkv_t[k][:, c.HD + m * 128:
                                              c.HD + (m + 1) * 128], xT, c.KL)
        if USE_FP8_SCORES:
            regroup(kt8, kT, c.KL)
        # V natural [klen-part, hd] (+ ones col per head for softmax denom)
        for jt in range(c.NTT):
            ps = psD.tile([128, 1024], F32, tag="d")
            for k in range(c.DPT):
                nc.tensor.matmul(
                    ps[:, 0:c.HD],
                    xT[:, k * c.KL + jt * 128: k * c.KL + (jt + 1) * 128],
                    qkv_t[k][:, 2 * c.HD: 3 * c.HD],
                    start=(k == 0), stop=(k == c.DPT - 1))
            dst = bass.AP(
                tensor=vb.tensor, offset=vb.offset + jt * VW,
                ap=[[c.NTT * VW, 128], [65, c.NHC], [1, c.DH]])
            nc.vector.tensor_copy(out=dst, in_=ps[:, 0:c.HD])
            ones = bass.AP(
                tensor=vb.tensor, offset=vb.offset + jt * VW + c.DH,
                ap=[[c.NTT * VW, 128], [65, c.NHC], [1, 1]])
            nc.vector.memset(ones, 1.0)
        # Q^T with scale and biases folded: rwq = SCALE*q + SCALE*rwb etc.
        for m in range(c.HPT):
            ps = psA.tile([128, 1024], F32, tag="a")
            for k in range(c.DPT):
                _mm512(nc, ps, qkv_t[k][:, ts(m, 128)],
                       lambda nb, ne, k=k: xT[:, k * c.KL + c.M + nb:
                                              k * c.KL + c.M + ne],
                       1024, start=(k == 0), stop=(k == c.DPT - 1))
            sl = ts(m, c.Q)
            nc.scalar.activation(out=rwq[:, sl], in_=ps[:],
                                 func=AF.Identity, bias=rwb_s[:, m:m + 1],
                                 scale=float(c.SCALE))
            nc.vector.tensor_scalar(out=rrq[:, sl], in0=ps[:],
                                    scalar1=rrb_s[:, m:m + 1],
                                    scalar2=float(c.SCALE),
                                    op0=ALU.mult, op1=ALU.add)
        if USE_FP8_SCORES:
            regroup(rwq8, rwq, c.Q)
            regroup(rrq8, rrq, c.Q)
        wqp.release()
        phX.release()
        wrp.release()
        phR.release()
        if USE_FP8_SCORES:
            stg.release()
        # ---- score-pipeline rings (allocated in the freed load space) ----
        # bdw: exp(BD) in absolute r-coords [0, 2048) + permanent zero tail
        # [2048, 2176) which realizes the causal mask (exp(-inf) = 0).
        sbp = tc.alloc_tile_pool(name="sbp", bufs=1)
        bdws, sbAs, sbBs = [], [], []
        for i in range(2):
            sbBs.append(sbp.tile([128, 2 * c.KL], BF16, tag="sbB%d" % i, name="sbB%d" % i))
            sbAs.append(sbp.tile([128, 2 * c.KL], BF16, tag="sbA%d" % i, name="sbA%d" % i))
        for i in range(3):
            b = sbp.tile([128, 2 * c.BDW], BF16, tag="bdw%d" % i, name="bdw%d" % i)
            nc.vector.memset(bass.AP(
                tensor=b.tensor, offset=b.offset + c.KL,
                ap=[[2 * c.BDW, 128], [c.BDW, 2], [1, 128]]), 0.0)
            bdws.append(b)

        # ============ phase 3: attention (exp-split) ============
        ptp = tc.alloc_tile_pool(name="ptp", bufs=2)
        ow_t = []
        for p in range(c.HPT):
            t = owp.tile([128, c.D], BF16, tag="ow_%d" % p)
            nc.gpsimd.dma_start(out=t[:], in_=io["oww"][ts(p, 128), :])
            ow_t.append(t)
        pend = []

        def score_mm(out_ap, q8, k8, hp, t2, s, koff, w):
            """fp8 DoubleRow score matmul: contraction 64 as [32, 2, .]."""
            tl_q, tl_k = q8[hp], k8[hp]
            hb = 32 * t2
            nc.tensor.matmul(
                out_ap,
                bass.AP(tensor=tl_q.tensor,
                        offset=tl_q.offset + hb * 2 * c.Q + s * 128,
                        ap=[[2 * c.Q, 32], [c.Q, 2], [1, 128]]),
                bass.AP(tensor=tl_k.tensor,
                        offset=tl_k.offset + hb * 2 * c.KL + koff,
                        ap=[[2 * c.KL, 32], [c.KL, 2], [1, w]]),
                start=True, stop=True, perf_mode=DR)

        def score_mm_bf16(out_ap, qst, kst, hp, t2, s, koff, w):
            hr = t2 * 64
            nc.tensor.matmul(
                out_ap,
                qst[hr:hr + 64, hp * c.Q + s * 128: hp * c.Q + (s + 1) * 128],
                kst[hr:hr + 64, hp * c.KL + koff: hp * c.KL + koff + w],
                start=True, stop=True)

        def score_iter(hp, s, pT_A, pT_B):
            """BD + AC pair-chunks with immediate Act exp drains."""
            jmx = c.jm(s)
            wst = c.Q - 128 * (s + 1)
            it = hp * c.NS + s
            slot = it % 2
            bdwt, sba = bdws[it % 3], sbAs[slot]
            for lo in range(0, jmx, 512):
                w = min(512, jmx - lo)
                ps = psD.tile([128, 1024], F32, tag="d")
                for t2 in range(2):
                    if USE_FP8_SCORES:
                        score_mm(ps[:, t2 * 512: t2 * 512 + w], rrq8, rtp8,
                                 hp, t2, s, wst + lo, w)
                    else:
                        score_mm_bf16(ps[:, t2 * 512: t2 * 512 + w], rrq, rTp,
                                      hp, t2, s, wst + lo, w)
                nc.scalar.activation(
                    out=bass.AP(tensor=bdwt.tensor,
                                offset=bdwt.offset + wst + lo,
                                ap=[[2 * c.BDW, 128], [c.BDW, 2], [1, w]]),
                    in_=bass.AP(tensor=ps.tensor, offset=ps.offset,
                                ap=[[1024, 128], [512, 2], [1, w]]),
                    func=AF.Exp)
            for lo in range(0, jmx, 512):
                w = min(512, jmx - lo)
                ps = psA.tile([128, 1024], F32, tag="a")
                for t2 in range(2):
                    if USE_FP8_SCORES:
                        score_mm(ps[:, t2 * 512: t2 * 512 + w], rwq8, kt8,
                                 hp, t2, s, lo, w)
                    else:
                        score_mm_bf16(ps[:, t2 * 512: t2 * 512 + w], rwq, kT,
                                      hp, t2, s, lo, w)
                nc.scalar.activation(
                    out=bass.AP(tensor=sba.tensor, offset=sba.offset + lo,
                                ap=[[2 * c.KL, 128], [c.KL, 2], [1, w]]),
                    in_=bass.AP(tensor=ps.tensor, offset=ps.offset,
                                ap=[[1024, 128], [512, 2], [1, w]]),
                    func=AF.Exp)
            pend.append((s, slot, it % 3))

        pend_tp = []

        def flush_dm():
            """rel-shift diag copy + exp-product for the oldest pending s."""
            s, slot, bslot = pend.pop(0)
            jmx = c.jm(s)
            wst = c.Q - 128 * (s + 1)
            bdwt, sba, sbb = bdws[bslot], sbAs[slot], sbBs[slot]
            # rel-shift: sbB[p, h2, j] = exp(BD)[p, h2, wst + 127 - p + j]
            nc.gpsimd.dma_start(
                out=bass.AP(tensor=sbb.tensor, offset=sbb.offset,
                            ap=[[2 * c.KL, 128], [c.KL, 2], [1, jmx]]),
                in_=bass.AP(tensor=bdwt.tensor,
                            offset=bdwt.offset + wst + 127,
                            ap=[[2 * c.BDW - 1, 128], [c.BDW, 2], [1, jmx]]))
            # p = exp(AC) * exp(BD)_shifted  (DVE 2x, in place into sbB)
            b3 = bass.AP(tensor=sbb.tensor, offset=sbb.offset,
                         ap=[[2 * c.KL, 128], [c.KL, 2], [1, jmx]])
            nc.vector.tensor_tensor(
                out=b3,
                in0=bass.AP(tensor=sba.tensor, offset=sba.offset,
                            ap=[[2 * c.KL, 128], [c.KL, 2], [1, jmx]]),
                in1=b3, op=ALU.mult)
            pend_tp.append((s, slot))

        def flush_tp(pT_A, pT_B):
            s, slot = pend_tp.pop(0)
            jmx = c.jm(s)
            sbb = sbBs[slot]
            for t2, pT in ((0, pT_A), (1, pT_B)):
                nc.sync.dma_start(
                    out=bass.AP(tensor=pT.tensor, offset=pT.offset + s * 128,
                                ap=[[c.NTT * c.Q, 128], [c.Q, jmx // 128], [1, 128]]),
                    in_=bass.AP(tensor=sbb.tensor,
                                offset=sbb.offset + t2 * c.KL,
                                ap=[[2 * c.KL, 128], [1, jmx]]),
                    transpose=True)

        def issue_pv(hp, t2, pT):
            h = 2 * hp + t2
            hr = t2 * 64
            ps = psA.tile([128, 1024], F32, tag="a")
            for c2 in range(2):
                lo, hi = c2 * 512, (c2 + 1) * 512
                njt = 12 if c2 == 0 else 16
                out = ps[0:65, c2 * 512:(c2 + 1) * 512]
                for jt in range(njt):
                    nlo = max(lo, 128 * (jt - 8))
                    nc.tensor.matmul(
                        out[:, nlo - lo:512],
                        vb[:, jt * VW + h * 65: jt * VW + h * 65 + 65],
                        pT[:, jt * c.Q + nlo: jt * c.Q + hi],
                        start=(jt == 0), stop=(jt == njt - 1))
            for c2 in range(2):
                sl = slice(c2 * 512, (c2 + 1) * 512)
                rd = small.tile([1, 512], BF16, tag="rd")
                with nc.allow_low_precision(reason="softmax denom recip"):
                    nc.vector.reciprocal(out=rd[:], in_=ps[64:65, sl])
                rdb = small.tile([64, 512], BF16, tag="rdb")
                nc.gpsimd.partition_broadcast(rdb[:], rd[:])
                nc.vector.tensor_tensor(
                    out=attnT[hr:hr + 64, hp * c.Q + c2 * 512:
                              hp * c.Q + (c2 + 1) * 512],
                    in0=ps[0:64, sl], in1=rdb[:], op=ALU.mult)

        prev = None
        for hp in range(c.NHC // 2):
            pT_A = pT_B = None
            for s in range(c.NS):
                score_iter(hp, s, None, None)
                if prev is not None and s in (2, 3):
                    issue_pv(prev[0], s - 2, prev[s - 1])
                if s == 3:
                    pT_A = ptp.tile([128, c.NTT * c.Q], BF16, tag="pT",
                                    name="pTA")
                    pT_B = ptp.tile([128, c.NTT * c.Q], BF16, tag="pT",
                                    name="pTB")
                # transposes (readers of sbB slot s-3) must precede the
                # diag copy of s-1 (writer of the same slot)
                if len(pend_tp) > 1:
                    flush_tp(pT_A, pT_B)
                if len(pend) > 1:
                    flush_dm()
            flush_tp(pT_A, pT_B)   # tp(4)
            flush_tp(pT_A, pT_B)   # tp(5)
            flush_dm()             # dm(7)
            while pend_tp:
                flush_tp(pT_A, pT_B)
            prev = (hp, pT_A, pT_B)
        issue_pv(prev[0], 0, prev[1])
        issue_pv(prev[0], 1, prev[2])

        ptp.release()
        sbp.release()
        if not USE_FP8_SCORES:
            stg.release()
        attk.release()

        # ============ phase 4: o_proj (natural out) -> ReduceScatter ============
        with tc.tile_pool(name="stO", bufs=3) as stage:
            for qb in range(c.NS):
                ost = stage.tile([128, c.D], BF16, tag="ost")
                for half in range(2):
                    ps = psD.tile([128, 1024], F32, tag="d")
                    for k in range(c.HPT):
                        nc.tensor.matmul(
                            ps[:, 0:512],
                            attnT[:, k * c.Q + qb * 128: k * c.Q + (qb + 1) * 128],
                            ow_t[k][:, half * 512: (half + 1) * 512],
                            start=(k == 0), stop=(k == c.HPT - 1))
                    nc.scalar.activation(out=ost[:, half * 512:(half + 1) * 512],
                                         in_=ps[:, 0:512], func=AF.Copy)
                nc.sync.dma_start(out=io["rs_bin"][ts(qb, 128), :], in_=ost[:])
        owp.release()
        atp.release()
        psD.release()
        psA.release()

        # ============ phase 5: LN1 + FFN + LN2 ============
        w1p = ctx.enter_context(tc.tile_pool(name="w1p", bufs=1))
        w1_t = []
        for k in range(c.DPT):
            t = w1p.tile([128, c.DI], BF16, tag="w1_%d" % k)
            nc.gpsimd.dma_start(out=t[:], in_=io["ffw1"][ts(k, 128), :])
            w1_t.append(t)

        # 8 single-bank accumulators for the k-outer FFN2 (and FFN1/LN use)
        psF = ctx.enter_context(tc.tile_pool(name="psF", bufs=1, space="PSUM"))

        def psf(i, shape, dtype=F32):
            return psF.tile(shape, dtype, tag="p%d" % (i % 8),
                            name="psf%d" % (i % 8))

        phE = ctx.enter_context(tc.tile_pool(name="phE", bufs=1))
        eps_t = phE.tile([128, 1], F32, tag="eps")
        nc.vector.memset(eps_t[:], c.LN_EPS)
        lns = {}
        for nm in ("ln1g", "ln1b", "ln2g", "ln2b"):
            tl = phE.tile([128, c.D], F32, tag=nm)
            bcast = bass.AP(tensor=io[nm].ap().tensor, offset=0,
                            ap=[[0, 128], [1, c.D]])
            nc.sync.dma_start(out=tl[:], in_=bcast)
            lns[nm] = tl
        fb1 = phE.tile([128, c.DI // 128], F32, tag="fb1")
        nc.sync.dma_start(out=fb1[:], in_=bass.AP(
            tensor=io["ffb1"].ap().tensor, offset=0, ap=[[1, 128], [128, c.DI // 128]]))
        fb2n = phE.tile([128, c.D], F32, tag="fb2n")
        nc.sync.dma_start(out=fb2n[:], in_=bass.AP(
            tensor=io["ffb2"].ap().tensor, offset=0, ap=[[0, 128], [1, c.D]]))

        ntt = c.TOKF // 128  # 4
        ffn = ctx.enter_context(tc.tile_pool(name="ffn", bufs=1))
        ln1r = ffn.tile([128, ntt * c.D], BF16, tag="ln1r")  # ln1 out + b2
        lnT = ffn.tile([128, c.DPT * c.TOKF], BF16, tag="lnT")
        hT = ffn.tile([128, (c.DI // 128) * c.TOKF], BF16, tag="hT")
        wres4 = ffn.tile([128, ntt * c.D], BF16, tag="wres4")
        for tt in range(ntt):
            nc.gpsimd.dma_start(out=wres4[:, ts(tt, c.D)],
                                in_=io["wres"][ts(tt, 128), :])

        if collective:
            nc.gpsimd.collective_compute(
                "ReduceScatter", ALU.add, replica_groups=rg,
                ins=[io["rs_bin"].ap().opt()], outs=[io["rs_bout"].ap().opt()])
        else:
            nc.sync.dma_start(out=io["rs_bout"].ap().opt(),
                              in_=io["rs_bin"].ap()[0:c.TOKF, :].opt())

        with tc.tile_pool(name="stE", bufs=2) as stage, \
             tc.tile_pool(name="w2s", bufs=8) as w2s:
            for tt in range(ntt):
                zb = stage.tile([128, c.D], BF16, tag="zb")
                nc.sync.dma_start(out=zb[:], in_=io["rs_bout"][ts(tt, 128), :])
                z = stage.tile([128, c.D], F32, tag="z")
                nc.vector.tensor_tensor(out=z[:], in0=wres4[:, ts(tt, c.D)],
                                        in1=zb[:], op=ALU.add)
                lsl = slice(tt * c.D, (tt + 1) * c.D)
                _layernorm_nat(nc, c, small, z[:], eps_t,
                               lns["ln1g"], lns["ln1b"], ln1r[:, lsl])
                znb = stage.tile([128, c.D], BF16, tag="znb")
                nc.vector.tensor_copy(out=znb[:], in_=ln1r[:, lsl])
                dstap = bass.AP(
                    tensor=lnT.tensor, offset=lnT.offset + tt * 128,
                    ap=[[c.DPT * c.TOKF, 128], [c.TOKF, c.DPT], [1, 128]])
                nc.sync.dma_start(out=dstap, in_=znb[:], transpose=True)
                # pre-add b2 for the FFN2 residual
                nc.vector.tensor_tensor(out=ln1r[:, lsl], in0=ln1r[:, lsl],
                                        in1=fb2n[:], op=ALU.add)
            # FFN1: hT[di, tok], m-outer with resident w1; token-halves so
            # the first half starts after only 2 of 4 LN1 tiles
            for g in range(2):
                for m in range(c.DI // 128):
                    ps = psf(m, [128, 256])
                    for k in range(c.DPT):
                        nc.tensor.matmul(
                            ps[:], w1_t[k][:, ts(m, 128)],
                            lnT[:, k * c.TOKF + g * 256: k * c.TOKF + (g + 1) * 256],
                            start=(k == 0), stop=(k == c.DPT - 1))
                    nc.scalar.activation(
                        out=hT[:, m * c.TOKF + g * 256: m * c.TOKF + (g + 1) * 256],
                        in_=ps[:], func=AF.Relu, bias=fb1[:, m:m + 1])
            # FFN2: k-outer, streaming w2, natural out [tok, d]
            nkt = c.DI // 128
            acc = [psf(i, [128, 512]) for i in range(8)]
            for k in range(nkt):
                w2t = w2s.tile([128, c.D], BF16, tag="w2")
                nc.gpsimd.dma_start(out=w2t[:], in_=io["ffw2"][ts(k, 128), :])
                for tb in range(ntt):
                    for half in range(2):
                        nc.tensor.matmul(
                            acc[tb * 2 + half][:],
                            hT[:, k * c.TOKF + tb * 128: k * c.TOKF + (tb + 1) * 128],
                            w2t[:, half * 512:(half + 1) * 512],
                            start=(k == 0), stop=(k == nkt - 1))
            for tb in range(ntt):
                o2n = stage.tile([128, c.D], F32, tag="o2n")
                for half in range(2):
                    nc.vector.tensor_tensor(
                        out=o2n[:, half * 512:(half + 1) * 512],
                        in0=acc[tb * 2 + half][:],
                        in1=ln1r[:, tb * c.D + half * 512: tb * c.D + (half + 1) * 512],
                        op=ALU.add)
                fin = stage.tile([128, c.D], F32, tag="fin")
                _layernorm_nat(nc, c, small, o2n[:], eps_t,
                               lns["ln2g"], lns["ln2b"], fin[:])
                nc.sync.dma_start(out=io["out"][ts(tb, 128), :], in_=fin[:])


def _layernorm_nat(nc, c, small, z, eps_t, g, b, out_dst):
    """LayerNorm over the free axis of z [128, D] fp32."""
    BN_FMAX = nc.vector.BN_STATS_FMAX
    d = z.shape[-1]
    sub = math.gcd(BN_FMAX, d)
    nsub = d // sub
    zr = z.rearrange("p (n f) -> p n f", f=sub)
    stats = small.tile([128, nsub, nc.vector.BN_STATS_DIM], F32, tag="bnst")
    for i in range(nsub):
        nc.vector.bn_stats(out=stats[:, i, :], in_=zr[:, i, :])
    mv = small.tile([128, nc.vector.BN_AGGR_DIM], F32, tag="bnag")
    nc.vector.bn_aggr(out=mv[:], in_=stats[:])
    mean, var = mv[:, 0:1], mv[:, 1:2]
    nc.scalar.activation(out=var, in_=var, func=AF.Sqrt, bias=eps_t[:], scale=1.0)
    nc.vector.reciprocal(out=var, in_=var)
    nc.vector.tensor_scalar(out=out_dst, in0=z, scalar1=mean, scalar2=var,
                            op0=ALU.subtract, op1=ALU.mult)
    nc.vector.tensor_tensor(out=out_dst, in0=out_dst, in1=g[:, 0:d], op=ALU.mult)
    nc.vector.tensor_tensor(out=out_dst, in0=out_dst, in1=b[:, 0:d], op=ALU.add)


# ============================================================
# host-side sharding + entry point
# ============================================================

def shard_inputs(inputs, c: Cfg = None):
    c = c or Cfg()
    w = np.asarray(inputs["w"], np.float32)
    r = np.asarray(inputs["r"], np.float32)
    mems = np.asarray(inputs["mems"], np.float32)
    qkv_w = np.asarray(inputs["qkv_w"], np.float32)
    r_net_w = np.asarray(inputs["r_net_w"], np.float32)
    o_w = np.asarray(inputs["o_w"], np.float32)
    r_w_bias = np.asarray(inputs["r_w_bias"], np.float32).reshape(-1)
    r_r_bias = np.asarray(inputs["r_r_bias"], np.float32).reshape(-1)
    NHD = qkv_w.shape[1] // 3
    rt = np.ascontiguousarray(r[:, 0, :].T)
    in_maps = []
    for core in range(c.N_CORES):
        b, hh = core // 2, core % 2
        hsl = slice(hh * c.HD, (hh + 1) * c.HD)
        xt_c = np.concatenate([mems[:, b, :], w[:, b, :]], axis=0).T
        qkvw_c = np.concatenate([qkv_w[:, j * NHD + hh * c.HD:
                                       j * NHD + (hh + 1) * c.HD]
                                 for j in range(3)], axis=1)
        in_maps.append({
            "xt": np.ascontiguousarray(xt_c),
            "rt": rt,
            "qkvw": np.ascontiguousarray(qkvw_c),
            "rnetw": np.ascontiguousarray(r_net_w[:, hsl]),
            "oww": np.ascontiguousarray(o_w[hsl, :]),
            "rwb": np.ascontiguousarray(r_w_bias[hsl][None, :]),
            "rrb": np.ascontiguousarray(r_r_bias[hsl][None, :]),
            "ln1g": np.asarray(inputs["ln1_g"], np.float32).reshape(1, -1),
            "ln1b": np.asarray(inputs["ln1_b"], np.float32).reshape(1, -1),
            "ln2g": np.asarray(inputs["ln2_g"], np.float32).reshape(1, -1),
            "ln2b": np.asarray(inputs["ln2_b"], np.float32).reshape(1, -1),
            "ffw1": np.asarray(inputs["ff_w1"], np.float32),
            "ffb1": np.asarray(inputs["ff_b1"], np.float32).reshape(1, -1),
            "ffw2": np.asarray(inputs["ff_w2"], np.float32),
            "ffb2": np.asarray(inputs["ff_b2"], np.float32).reshape(1, -1),
            "wres": np.ascontiguousarray(w[hh * c.TOKF:(hh + 1) * c.TOKF, b, :]),
        })
    return in_maps


def unshard_output(results, inputs, c: Cfg = None):
    c = c or Cfg()
    w = np.asarray(inputs["w"])
    Q, B, D = w.shape
    out = np.zeros((Q, B, D), np.float32)
    for core in range(c.N_CORES):
        b, hh = core // 2, core % 2
        out[hh * c.TOKF:(hh + 1) * c.TOKF, b, :] = results[core]["out"]
    return out


_NC_CACHE = {}


def kernel(**inputs):
    if "nc" not in _NC_CACHE:
        _NC_CACHE["nc"] = build_kernel()
    nc = _NC_CACHE["nc"]
    in_maps = shard_inputs(inputs)
    from concourse.bass_utils import run_bass_kernel_spmd
    res = run_bass_kernel_spmd(nc, in_maps, core_ids=list(range(Cfg.N_CORES)))
    return unshard_output(res.results, inputs)


# revision 50
# speedup vs baseline: 1.0045x; 1.0000x over previous
"""Trainium2 Bass kernel for nn_MemTransformerLM (Transformer-XL layer).

Sharding (8 cores): batch (4) x head-half (2). Core c handles batch b = c//2
and heads [hh*8, hh*8+8), hh = c%2, for all 1024 queries. After o_proj a
2-rank ReduceScatter over core pairs (2b, 2b+1) splits tokens for the FFN:
even core keeps tokens [0,512), odd [512,1024).

v2 rewrite ("exp-split" attention):
 - host pre-transposes x/r -> direct [d-part, klen] loads, no on-chip
   input transposes.
 - softmax split: p = exp(AC) * exp(BD_shifted). Act exponentiates the
   AC and BD PSUM chunks directly into SBUF (no PSUM-drain copies), a
   single gpsimd DMA applies the Transformer-XL rel-shift as a diagonal
   copy, and DVE multiplies the two factors at 2x (all-bf16).
 - the causal/memory mask is a permanently-zeroed tail of the BD buffer
   (exp(-inf) = 0), so masking costs nothing per iteration.
 - PV softmax denominator via a ones-column in V; reciprocal broadcast
   uses gpsimd partition_broadcast instead of a DMA.
 - PV of pair hp-1 is issued at s=2/3 of pair hp (after the previous
   pair's transpose tail has landed) to keep PE fed without stalls.
"""

import contextlib
import math

import numpy as np

import concourse.bass as bass
import concourse.bacc as bacc
import concourse.mybir as mybir
import concourse.tile as tile

F32 = mybir.dt.float32
BF16 = mybir.dt.bfloat16
FP8 = mybir.dt.float8e4
DR = mybir.MatmulPerfMode.DoubleRow
AF = mybir.ActivationFunctionType
ALU = mybir.AluOpType

USE_FP8_SCORES = False


class Cfg:
    D = 1024      # model dim
    NHC = 8       # heads per core
    DH = 64       # head dim
    KL = 2048     # key length
    Q = 1024      # query length
    DI = 4096     # ffn inner
    LN_EPS = 1e-5
    N_CORES = 8

    HD = property(lambda s: s.NHC * s.DH)       # head dims per core (512)
    SCALE = property(lambda s: 1.0 / (s.DH ** 0.5))
    M = property(lambda s: s.KL - s.Q)          # mem length
    NS = property(lambda s: s.Q // 128)         # q tiles (8)
    DPT = property(lambda s: s.D // 128)        # 8
    HPT = property(lambda s: s.HD // 128)       # 4
    NTT = property(lambda s: s.KL // 128)       # 16
    BDW = property(lambda s: s.KL + 128)        # bd buffer width (2176)
    TOKF = property(lambda s: s.Q // 2)         # ffn tokens per core (512)

    def jm(self, s):
        """exact key extent for q-tile s: multiple of 128."""
        return 128 * (s + 9)


def _mm512(nc, ps, lhsT, rhs_fn, width, start, stop, lhsT_fn=None,
           perf_mode=None):
    """Issue width//512 matmuls of <=512 cols into ps[:, off:off+...]."""
    for nb in range(0, width, 512):
        ne = min(width, nb + 512)
        l = lhsT_fn(nb, ne) if lhsT_fn is not None else lhsT
        nc.tensor.matmul(ps[:, nb:ne], l, rhs_fn(nb, ne),
                         start=start, stop=stop, perf_mode=perf_mode)


def ts(i, n):
    return slice(i * n, (i + 1) * n)


def build_kernel(c: Cfg = None, collective=True, repeat=1):
    c = c or Cfg()
    nc = bacc.Bacc("TRN2", target_bir_lowering=False)

    io = {}
    def din(name, shape):
        io[name] = nc.dram_tensor(name, shape, F32, kind="ExternalInput")
    din("xt", [c.D, c.KL])        # pre-transposed concat(mems, w)
    din("rt", [c.D, c.KL])        # pre-transposed r
    din("qkvw", [c.D, 3 * c.HD])
    din("rnetw", [c.D, c.HD])
    din("oww", [c.HD, c.D])
    din("rwb", [1, c.HD])
    din("rrb", [1, c.HD])
    din("ln1g", [1, c.D]); din("ln1b", [1, c.D])
    din("ln2g", [1, c.D]); din("ln2b", [1, c.D])
    din("ffw1", [c.D, c.DI]); din("ffb1", [1, c.DI])
    din("ffw2", [c.DI, c.D]); din("ffb2", [1, c.D])
    din("wres", [c.TOKF, c.D])
    io["out"] = nc.dram_tensor("out", [c.TOKF, c.D], F32, kind="ExternalOutput")
    io["rs_bin"] = nc.dram_tensor("rs_bin", [c.Q, c.D], BF16)
    io["rs_bout"] = nc.dram_tensor("rs_bout", [c.TOKF, c.D], BF16)

    with tile.TileContext(nc) as tc:
        for _ in range(repeat):
            _body(tc, nc, c, io, collective=collective)
    nc.finalize()
    return nc


def _body(tc, nc, c, io, collective=True):
    ctx = contextlib.ExitStack()
    rg = [[i, i + 1] for i in range(0, c.N_CORES, 2)]
    with ctx:
        small = ctx.enter_context(tc.tile_pool(name="small", bufs=2))
        keep = ctx.enter_context(tc.tile_pool(name="keep", bufs=1))

        # manual pools, stack-disciplined: released in reverse alloc order,
        # with phR/wrp -> phX/wqp -> ptp pushed/popped in sequence so their
        # SBUF space is reused across phases.
        psA = tc.alloc_tile_pool(name="psA", bufs=2, space="PSUM")
        psD = tc.alloc_tile_pool(name="psD", bufs=2, space="PSUM")
        atp = tc.alloc_tile_pool(name="atp", bufs=1)
        owp = tc.alloc_tile_pool(name="owp", bufs=1)
        attk = tc.alloc_tile_pool(name="attk", bufs=1)
        stg = tc.alloc_tile_pool(name="stg", bufs=1)
        phR = tc.alloc_tile_pool(name="phR", bufs=1)
        wrp = tc.alloc_tile_pool(name="wrp", bufs=1)
        phX = tc.alloc_tile_pool(name="phX", bufs=1)
        wqp = tc.alloc_tile_pool(name="wqp", bufs=1)

        # ---- persistent small constants ----
        rwb_s = keep.tile([128, c.HPT], F32, tag="rwb")
        rrb_s = keep.tile([128, c.HPT], F32, tag="rrb")
        nc.sync.dma_start(out=rwb_s[:], in_=bass.AP(
            tensor=io["rwb"].ap().tensor, offset=0, ap=[[1, 128], [128, c.HPT]]))
        nc.sync.dma_start(out=rrb_s[:], in_=bass.AP(
            tensor=io["rrb"].ap().tensor, offset=0, ap=[[1, 128], [128, c.HPT]]))
        # fold the attention scale into the biases (applied at Q^T creation)
        nc.vector.tensor_scalar_mul(out=rwb_s[:], in0=rwb_s[:],
                                    scalar1=float(c.SCALE))
        nc.vector.tensor_scalar_mul(out=rrb_s[:], in0=rrb_s[:],
                                    scalar1=float(c.SCALE))


        # ============ phase 1: all input loads (gpsimd cast f32->bf16) =====
        rT = phR.tile([128, c.DPT * c.KL], BF16, tag="rT")
        xT = phX.tile([128, c.DPT * c.KL], BF16, tag="xT")
        wr_t, qkv_t = [], []
        for k in range(c.DPT):
            t = wrp.tile([128, c.HD], BF16, tag="wr_%d" % k)
            nc.gpsimd.dma_start(out=t[:], in_=io["rnetw"][ts(k, 128), :])
            wr_t.append(t)
            nc.gpsimd.dma_start(out=rT[:, ts(k, c.KL)],
                                in_=io["rt"][ts(k, 128), :])
        for k in range(c.DPT):
            t = wqp.tile([128, 3 * c.HD], BF16, tag="qkv_%d" % k)
            nc.gpsimd.dma_start(out=t[:], in_=io["qkvw"][ts(k, 128), :])
            qkv_t.append(t)
            nc.gpsimd.dma_start(out=xT[:, ts(k, c.KL)],
                                in_=io["xt"][ts(k, 128), :])

        def dr3(t, pair_w, w, off):
            """[K=128, 2, w] DoubleRow operand view of pair-tile t."""
            return bass.AP(tensor=t.tensor, offset=t.offset + off,
                           ap=[[2 * pair_w, 128], [pair_w, 2], [1, w]])

        # ============ phase 2: projections (fp8 DR, PSUM drains on DVE) ====
        VW = c.NHC * 65
        vb = attk.tile([128, c.NTT * VW], BF16, tag="vb")
        # fp8 split-layout score operands: one [64, 2*W] tile per head-pair;
        # head t2 at partition base 32*t2 with dh-halves adjacent in free dim.
        if USE_FP8_SCORES:
            kt8 = [attk.tile([64, 2 * c.KL], FP8, tag="kt8_%d" % i,
                             name="kt8_%d" % i) for i in range(c.HPT)]
            rtp8 = [attk.tile([64, 2 * c.KL], FP8, tag="rtp8_%d" % i,
                              name="rtp8_%d" % i) for i in range(c.HPT)]
            rwq8 = [attk.tile([64, 2 * c.Q], FP8, tag="rwq8_%d" % i,
                              name="rwq8_%d" % i) for i in range(c.HPT)]
            rrq8 = [attk.tile([64, 2 * c.Q], FP8, tag="rrq8_%d" % i,
                              name="rrq8_%d" % i) for i in range(c.HPT)]
        attnT = atp.tile([128, c.HPT * c.Q], BF16, tag="attnT")

        # bf16 staging for the projection outputs, released before attention.
        # rTp and kT share one tile (used sequentially around regroups).
        rTp = stg.tile([128, c.HPT * c.KL], BF16, tag="rTp")
        kT = rTp if USE_FP8_SCORES else stg.tile([128, c.HPT * c.KL], BF16,
                                                 tag="kT", name="kT")
        rwq = stg.tile([128, c.HPT * c.Q], BF16, tag="rwq")
        rrq = stg.tile([128, c.HPT * c.Q], BF16, tag="rrq")

        def regroup(dst_tiles, src, width):
            """bf16 [128, HPT*width] -> fp8 split layout (cast DMA)."""
            for hp in range(c.HPT):
                tl = dst_tiles[hp]
                for hh in range(2):
                    nc.gpsimd.dma_start(
                        out=bass.AP(tensor=tl.tensor,
                                    offset=tl.offset + hh * 32 * 2 * width,
                                    ap=[[2 * width, 32], [width, 2],
                                        [1, width]]),
                        in_=bass.AP(tensor=src.tensor,
                                    offset=src.offset + hp * width
                                    + hh * 64 * c.HPT * width,
                                    ap=[[c.HPT * width, 32],
                                        [32 * c.HPT * width, 2], [1, width]]))

        # rTp = (r @ r_net_w)^T  [hd-part, klen]; k-outer in two passes of
        # 4 psum chunks so the PE k-steps track the rT tile arrivals.
        def kouter_proj(dst, lhs_col_fn, rhs, out_cols):
            for half in range(2):
                chunks = [(m, ch) for m in (2 * half, 2 * half + 1)
                          for ch in range(2)]
                pss = {}
                for i, (m, ch) in enumerate(chunks):
                    pool = psA if i % 2 == 0 else psD
                    pss[(m, ch)] = pool.tile([128, 1024], F32,
                                             tag="a" if i % 2 == 0 else "d",
                                             name="ps_%d_%d" % (m, ch))
                for k in range(c.DPT):
                    for m, ch in chunks:
                        _mm512(nc, pss[(m, ch)], lhs_col_fn(k, m),
                               lambda nb, ne, k=k, ch=ch:
                                   rhs[:, k * c.KL + ch * 1024 + nb:
                                       k * c.KL + ch * 1024 + ne],
                               1024, start=(k == 0), stop=(k == c.DPT - 1))
                for m, ch in chunks:
                    nc.vector.tensor_copy(
                        out=dst[:, m * out_cols + ch * 1024:
                                m * out_cols + (ch + 1) * 1024],
                        in_=pss[(m, ch)][:])

        kouter_proj(rTp, lambda k, m: wr_t[k][:, ts(m, 128)], rT, c.KL)
        if USE_FP8_SCORES:
            regroup(rtp8, rTp, c.KL)


        # K^T [hd-part, klen]
        kouter_proj(kT, lambda k, m: qkv_t[k][:, c.HD + m * 128:
                                              c.HD + (m + 1) * 128], xT, c.KL)
        if USE_FP8_SCORES:
            regroup(kt8, kT, c.KL)
        # V natural [klen-part, hd] (+ ones col per head for softmax denom)
        for jt in range(c.NTT):
            ps = psD.tile([128, 1024], F32, tag="d")
            for k in range(c.DPT):
                nc.tensor.matmul(
                    ps[:, 0:c.HD],
                    xT[:, k * c.KL + jt * 128: k * c.KL + (jt + 1) * 128],
                    qkv_t[k][:, 2 * c.HD: 3 * c.HD],
                    start=(k == 0), stop=(k == c.DPT - 1))
            dst = bass.AP(
                tensor=vb.tensor, offset=vb.offset + jt * VW,
                ap=[[c.NTT * VW, 128], [65, c.NHC], [1, c.DH]])
            nc.vector.tensor_copy(out=dst, in_=ps[:, 0:c.HD])
            ones = bass.AP(
                tensor=vb.tensor, offset=vb.offset + jt * VW + c.DH,
                ap=[[c.NTT * VW, 128], [65, c.NHC], [1, 1]])
            nc.vector.memset(ones, 1.0)
        # Q^T with scale and biases folded: rwq = SCALE*q + SCALE*rwb etc.
        for m in range(c.HPT):
            ps = psA.tile([128, 1024], F32, tag="a")
            for k in range(c.DPT):
                _mm512(nc, ps, qkv_t[k][:, ts(m, 128)],
                       lambda nb, ne, k=k: xT[:, k * c.KL + c.M + nb:
                                              k * c.KL + c.M + ne],
                       1024, start=(k == 0), stop=(k == c.DPT - 1))
            sl = ts(m, c.Q)
            nc.scalar.activation(out=rwq[:, sl], in_=ps[:],
                                 func=AF.Identity, bias=rwb_s[:, m:m + 1],
                                 scale=float(c.SCALE))
            nc.vector.tensor_scalar(out=rrq[:, sl], in0=ps[:],
                                    scalar1=rrb_s[:, m:m + 1],
                                    scalar2=float(c.SCALE),
                                    op0=ALU.mult, op1=ALU.add)
        if USE_FP8_SCORES:
            regroup(rwq8, rwq, c.Q)
            regroup(rrq8, rrq, c.Q)
        wqp.release()
        phX.release()
        wrp.release()
        phR.release()
        if USE_FP8_SCORES:
            stg.release()
        # ---- score-pipeline rings (allocated in the freed load space) ----
        # bdw: exp(BD) in absolute r-coords [0, 2048) + permanent zero tail
        # [2048, 2176) which realizes the causal mask (exp(-inf) = 0).
        sbp = tc.alloc_tile_pool(name="sbp", bufs=1)
        bdws, sbAs, sbBs = [], [], []
        for i in range(2):
            sbBs.append(sbp.tile([128, 2 * c.KL], BF16, tag="sbB%d" % i, name="sbB%d" % i))
            sbAs.append(sbp.tile([128, 2 * c.KL], BF16, tag="sbA%d" % i, name="sbA%d" % i))
        for i in range(3):
            b = sbp.tile([128, 2 * c.BDW], BF16, tag="bdw%d" % i, name="bdw%d" % i)
            nc.vector.memset(bass.AP(
                tensor=b.tensor, offset=b.offset + c.KL,
                ap=[[2 * c.BDW, 128], [c.BDW, 2], [1, 128]]), 0.0)
            bdws.append(b)

        # ============ phase 3: attention (exp-split) ============
        ptp = tc.alloc_tile_pool(name="ptp", bufs=2)
        ow_t = []
        for p in range(c.HPT):
            t = owp.tile([128, c.D], BF16, tag="ow_%d" % p)
            nc.gpsimd.dma_start(out=t[:], in_=io["oww"][ts(p, 128), :])
            ow_t.append(t)
        pend = []

        def score_mm(out_ap, q8, k8, hp, t2, s, koff, w):
            """fp8 DoubleRow score matmul: contraction 64 as [32, 2, .]."""
            tl_q, tl_k = q8[hp], k8[hp]
            hb = 32 * t2
            nc.tensor.matmul(
                out_ap,
                bass.AP(tensor=tl_q.tensor,
                        offset=tl_q.offset + hb * 2 * c.Q + s * 128,
                        ap=[[2 * c.Q, 32], [c.Q, 2], [1, 128]]),
                bass.AP(tensor=tl_k.tensor,
                        offset=tl_k.offset + hb * 2 * c.KL + koff,
                        ap=[[2 * c.KL, 32], [c.KL, 2], [1, w]]),
                start=True, stop=True, perf_mode=DR)

        def score_mm_bf16(out_ap, qst, kst, hp, t2, s, koff, w):
            hr = t2 * 64
            nc.tensor.matmul(
                out_ap,
                qst[hr:hr + 64, hp * c.Q + s * 128: hp * c.Q + (s + 1) * 128],
                kst[hr:hr + 64, hp * c.KL + koff: hp * c.KL + koff + w],
                start=True, stop=True)

        def score_iter(hp, s, pT_A, pT_B):
            """BD + AC pair-chunks with immediate Act exp drains."""
            jmx = c.jm(s)
            wst = c.Q - 128 * (s + 1)
            it = hp * c.NS + s
            slot = it % 2
            bdwt, sba = bdws[it % 3], sbAs[slot]
            for lo in range(0, jmx, 512):
                w = min(512, jmx - lo)
                ps = psD.tile([128, 1024], F32, tag="d")
                for t2 in range(2):
                    if USE_FP8_SCORES:
                        score_mm(ps[:, t2 * 512: t2 * 512 + w], rrq8, rtp8,
                                 hp, t2, s, wst + lo, w)
                    else:
                        score_mm_bf16(ps[:, t2 * 512: t2 * 512 + w], rrq, rTp,
                                      hp, t2, s, wst + lo, w)
                nc.scalar.activation(
                    out=bass.AP(tensor=bdwt.tensor,
                                offset=bdwt.offset + wst + lo,
                                ap=[[2 * c.BDW, 128], [c.BDW, 2], [1, w]]),
                    in_=bass.AP(tensor=ps.tensor, offset=ps.offset,
                                ap=[[1024, 128], [512, 2], [1, w]]),
                    func=AF.Exp)
            for lo in range(0, jmx, 512):
                w = min(512, jmx - lo)
                ps = psA.tile([128, 1024], F32, tag="a")
                for t2 in range(2):
                    if USE_FP8_SCORES:
                        score_mm(ps[:, t2 * 512: t2 * 512 + w], rwq8, kt8,
                                 hp, t2, s, lo, w)
                    else:
                        score_mm_bf16(ps[:, t2 * 512: t2 * 512 + w], rwq, kT,
                                      hp, t2, s, lo, w)
                nc.scalar.activation(
                    out=bass.AP(tensor=sba.tensor, offset=sba.offset + lo,
                                ap=[[2 * c.KL, 128], [c.KL, 2], [1, w]]),
                    in_=bass.AP(tensor=ps.tensor, offset=ps.offset,
                                ap=[[1024, 128], [512, 2], [1, w]]),
                    func=AF.Exp)
            pend.append((s, slot, it % 3))

        pend_tp = []

        def flush_dm():
            """rel-shift diag copy + exp-product for the oldest pending s."""
            s, slot, bslot = pend.pop(0)
            jmx = c.jm(s)
            wst = c.Q - 128 * (s + 1)
            bdwt, sba, sbb = bdws[bslot], sbAs[slot], sbBs[slot]
            # rel-shift: sbB[p, h2, j] = exp(BD)[p, h2, wst + 127 - p + j]
            nc.gpsimd.dma_start(
                out=bass.AP(tensor=sbb.tensor, offset=sbb.offset,
                            ap=[[2 * c.KL, 128], [c.KL, 2], [1, jmx]]),
                in_=bass.AP(tensor=bdwt.tensor,
                            offset=bdwt.offset + wst + 127,
                            ap=[[2 * c.BDW - 1, 128], [c.BDW, 2], [1, jmx]]))
            # p = exp(AC) * exp(BD)_shifted  (DVE 2x, in place into sbB)
            b3 = bass.AP(tensor=sbb.tensor, offset=sbb.offset,
                         ap=[[2 * c.KL, 128], [c.KL, 2], [1, jmx]])
            nc.vector.tensor_tensor(
                out=b3,
                in0=bass.AP(tensor=sba.tensor, offset=sba.offset,
                            ap=[[2 * c.KL, 128], [c.KL, 2], [1, jmx]]),
                in1=b3, op=ALU.mult)
            pend_tp.append((s, slot))

        def tp_half(s, slot, t2, pT):
            jmx = c.jm(s)
            sbb = sbBs[slot]
            nc.sync.dma_start(
                out=bass.AP(tensor=pT.tensor, offset=pT.offset + s * 128,
                            ap=[[c.NTT * c.Q, 128], [c.Q, jmx // 128], [1, 128]]),
                in_=bass.AP(tensor=sbb.tensor,
                            offset=sbb.offset + t2 * c.KL,
                            ap=[[2 * c.KL, 128], [1, jmx]]),
                transpose=True)

        def flush_tp(pT_A, pT_B):
            s, slot = pend_tp.pop(0)
            tp_half(s, slot, 0, pT_A)
            tp_half(s, slot, 1, pT_B)

        def issue_pv(hp, t2, pT):
            h = 2 * hp + t2
            hr = t2 * 64
            ps = psA.tile([128, 1024], F32, tag="a")
            for c2 in range(2):
                lo, hi = c2 * 512, (c2 + 1) * 512
                njt = 12 if c2 == 0 else 16
                out = ps[0:65, c2 * 512:(c2 + 1) * 512]
                for jt in range(njt):
                    nlo = max(lo, 128 * (jt - 8))
                    nc.tensor.matmul(
                        out[:, nlo - lo:512],
                        vb[:, jt * VW + h * 65: jt * VW + h * 65 + 65],
                        pT[:, jt * c.Q + nlo: jt * c.Q + hi],
                        start=(jt == 0), stop=(jt == njt - 1))
            for c2 in range(2):
                sl = slice(c2 * 512, (c2 + 1) * 512)
                rd = small.tile([1, 512], BF16, tag="rd")
                with nc.allow_low_precision(reason="softmax denom recip"):
                    nc.vector.reciprocal(out=rd[:], in_=ps[64:65, sl])
                rdb = small.tile([64, 512], BF16, tag="rdb")
                nc.gpsimd.partition_broadcast(rdb[:], rd[:])
                nc.vector.tensor_tensor(
                    out=attnT[hr:hr + 64, hp * c.Q + c2 * 512:
                              hp * c.Q + (c2 + 1) * 512],
                    in0=ps[0:64, sl], in1=rdb[:], op=ALU.mult)

        prev = None
        for hp in range(c.NHC // 2):
            pT_A = pT_B = None
            for s in range(c.NS):
                score_iter(hp, s, None, None)
                if prev is not None and s in (2, 3):
                    issue_pv(prev[0], s - 2, prev[s - 1])
                if s == 3:
                    pT_A = ptp.tile([128, c.NTT * c.Q], BF16, tag="pT",
                                    name="pTA")
                    pT_B = ptp.tile([128, c.NTT * c.Q], BF16, tag="pT",
                                    name="pTB")
                # transposes (readers of sbB slot s-3) must precede the
                # diag copy of s-1 (writer of the same slot)
                if len(pend_tp) > 1:
                    flush_tp(pT_A, pT_B)
                if len(pend) > 1:
                    flush_dm()
            flush_tp(pT_A, pT_B)   # tp(4)
            flush_tp(pT_A, pT_B)   # tp(5)
            flush_dm()             # dm(7)
            tail = list(pend_tp)
            pend_tp.clear()
            for s2, sl2 in tail:
                tp_half(s2, sl2, 0, pT_A)
            for s2, sl2 in tail:
                tp_half(s2, sl2, 1, pT_B)
            prev = (hp, pT_A, pT_B)
        issue_pv(prev[0], 0, prev[1])
        issue_pv(prev[0], 1, prev[2])

        ptp.release()
        sbp.release()
        if not USE_FP8_SCORES:
            stg.release()
        attk.release()

        # ============ phase 4: o_proj (natural out) -> ReduceScatter ============
        with tc.tile_pool(name="stO", bufs=3) as stage:
            for qb in range(c.NS):
                ost = stage.tile([128, c.D], BF16, tag="ost")
                for half in range(2):
                    pool, tg = (psD, "d") if half == 0 else (psA, "a")
                    ps = pool.tile([128, 1024], F32, tag=tg)
                    for k in range(c.HPT):
                        nc.tensor.matmul(
                            ps[:, 0:512],
                            attnT[:, k * c.Q + qb * 128: k * c.Q + (qb + 1) * 128],
                            ow_t[k][:, half * 512: (half + 1) * 512],
                            start=(k == 0), stop=(k == c.HPT - 1))
                    nc.scalar.activation(out=ost[:, half * 512:(half + 1) * 512],
                                         in_=ps[:, 0:512], func=AF.Copy)
                nc.sync.dma_start(out=io["rs_bin"][ts(qb, 128), :], in_=ost[:])
        owp.release()
        atp.release()
        psD.release()
        psA.release()

        # ============ phase 5: LN1 + FFN + LN2 ============
        w1p = ctx.enter_context(tc.tile_pool(name="w1p", bufs=1))
        w1_t = []
        for k in range(c.DPT):
            t = w1p.tile([128, c.DI], BF16, tag="w1_%d" % k)
            nc.gpsimd.dma_start(out=t[:], in_=io["ffw1"][ts(k, 128), :])
            w1_t.append(t)

        # 8 single-bank accumulators for the k-outer FFN2 (and FFN1/LN use)
        psF = ctx.enter_context(tc.tile_pool(name="psF", bufs=1, space="PSUM"))

        def psf(i, shape, dtype=F32):
            return psF.tile(shape, dtype, tag="p%d" % (i % 8),
                            name="psf%d" % (i % 8))

        phE = ctx.enter_context(tc.tile_pool(name="phE", bufs=1))
        eps_t = phE.tile([128, 1], F32, tag="eps")
        nc.vector.memset(eps_t[:], c.LN_EPS)
        lns = {}
        for nm in ("ln1g", "ln1b", "ln2g", "ln2b"):
            tl = phE.tile([128, c.D], F32, tag=nm)
            bcast = bass.AP(tensor=io[nm].ap().tensor, offset=0,
                            ap=[[0, 128], [1, c.D]])
            nc.sync.dma_start(out=tl[:], in_=bcast)
            lns[nm] = tl
        fb1 = phE.tile([128, c.DI // 128], F32, tag="fb1")
        nc.sync.dma_start(out=fb1[:], in_=bass.AP(
            tensor=io["ffb1"].ap().tensor, offset=0, ap=[[1, 128], [128, c.DI // 128]]))
        fb2n = phE.tile([128, c.D], F32, tag="fb2n")
        nc.sync.dma_start(out=fb2n[:], in_=bass.AP(
            tensor=io["ffb2"].ap().tensor, offset=0, ap=[[0, 128], [1, c.D]]))

        ntt = c.TOKF // 128  # 4
        ffn = ctx.enter_context(tc.tile_pool(name="ffn", bufs=1))
        ln1r = ffn.tile([128, ntt * c.D], BF16, tag="ln1r")  # ln1 out + b2
        lnT = ffn.tile([128, c.DPT * c.TOKF], BF16, tag="lnT")
        hT = ffn.tile([128, (c.DI // 128) * c.TOKF], BF16, tag="hT")
        wres4 = ffn.tile([128, ntt * c.D], BF16, tag="wres4")
        for tt in range(ntt):
            nc.gpsimd.dma_start(out=wres4[:, ts(tt, c.D)],
                                in_=io["wres"][ts(tt, 128), :])

        if collective:
            nc.gpsimd.collective_compute(
                "ReduceScatter", ALU.add, replica_groups=rg,
                ins=[io["rs_bin"].ap().opt()], outs=[io["rs_bout"].ap().opt()])
        else:
            nc.sync.dma_start(out=io["rs_bout"].ap().opt(),
                              in_=io["rs_bin"].ap()[0:c.TOKF, :].opt())

        with tc.tile_pool(name="stE", bufs=2) as stage, \
             tc.tile_pool(name="w2s", bufs=8) as w2s:
            for tt in range(ntt):
                zb = stage.tile([128, c.D], BF16, tag="zb")
                nc.sync.dma_start(out=zb[:], in_=io["rs_bout"][ts(tt, 128), :])
                z = stage.tile([128, c.D], F32, tag="z")
                nc.vector.tensor_tensor(out=z[:], in0=wres4[:, ts(tt, c.D)],
                                        in1=zb[:], op=ALU.add)
                lsl = slice(tt * c.D, (tt + 1) * c.D)
                _layernorm_nat(nc, c, small, z[:], eps_t,
                               lns["ln1g"], lns["ln1b"], ln1r[:, lsl])
                znb = stage.tile([128, c.D], BF16, tag="znb")
                nc.vector.tensor_copy(out=znb[:], in_=ln1r[:, lsl])
                dstap = bass.AP(
                    tensor=lnT.tensor, offset=lnT.offset + tt * 128,
                    ap=[[c.DPT * c.TOKF, 128], [c.TOKF, c.DPT], [1, 128]])
                nc.sync.dma_start(out=dstap, in_=znb[:], transpose=True)
                # pre-add b2 for the FFN2 residual
                nc.vector.tensor_tensor(out=ln1r[:, lsl], in0=ln1r[:, lsl],
                                        in1=fb2n[:], op=ALU.add)
            # FFN1: hT[di, tok], m-outer with resident w1; token-halves so
            # the first half starts after only 2 of 4 LN1 tiles
            for g in range(2):
                for m in range(c.DI // 128):
                    ps = psf(m, [128, 256])
                    for k in range(c.DPT):
                        nc.tensor.matmul(
                            ps[:], w1_t[k][:, ts(m, 128)],
                            lnT[:, k * c.TOKF + g * 256: k * c.TOKF + (g + 1) * 256],
                            start=(k == 0), stop=(k == c.DPT - 1))
                    nc.scalar.activation(
                        out=hT[:, m * c.TOKF + g * 256: m * c.TOKF + (g + 1) * 256],
                        in_=ps[:], func=AF.Relu, bias=fb1[:, m:m + 1])
            # FFN2: k-outer, streaming w2, natural out [tok, d]
            nkt = c.DI // 128
            acc = [psf(i, [128, 512]) for i in range(8)]
            for k in range(nkt):
                w2t = w2s.tile([128, c.D], BF16, tag="w2")
                nc.gpsimd.dma_start(out=w2t[:], in_=io["ffw2"][ts(k, 128), :])
                for tb in range(ntt):
                    for half in range(2):
                        nc.tensor.matmul(
                            acc[tb * 2 + half][:],
                            hT[:, k * c.TOKF + tb * 128: k * c.TOKF + (tb + 1) * 128],
                            w2t[:, half * 512:(half + 1) * 512],
                            start=(k == 0), stop=(k == nkt - 1))
            for tb in range(ntt):
                o2n = stage.tile([128, c.D], F32, tag="o2n")
                for half in range(2):
                    nc.vector.tensor_tensor(
                        out=o2n[:, half * 512:(half + 1) * 512],
                        in0=acc[tb * 2 + half][:],
                        in1=ln1r[:, tb * c.D + half * 512: tb * c.D + (half + 1) * 512],
                        op=ALU.add)
                fin = stage.tile([128, c.D], F32, tag="fin")
                _layernorm_nat(nc, c, small, o2n[:], eps_t,
                               lns["ln2g"], lns["ln2b"], fin[:])
                nc.sync.dma_start(out=io["out"][ts(tb, 128), :], in_=fin[:])


def _layernorm_nat(nc, c, small, z, eps_t, g, b, out_dst):
    """LayerNorm over the free axis of z [128, D] fp32."""
    BN_FMAX = nc.vector.BN_STATS_FMAX
    d = z.shape[-1]
    sub = math.gcd(BN_FMAX, d)
    nsub = d // sub
    zr = z.rearrange("p (n f) -> p n f", f=sub)
    stats = small.tile([128, nsub, nc.vector.BN_STATS_DIM], F32, tag="bnst")
    for i in range(nsub):
        nc.vector.bn_stats(out=stats[:, i, :], in_=zr[:, i, :])
    mv = small.tile([128, nc.vector.BN_AGGR_DIM], F32, tag="bnag")
    nc.vector.bn_aggr(out=mv[:], in_=stats[:])
    mean, var = mv[:, 0:1], mv[:, 1:2]
    nc.scalar.activation(out=var, in_=var, func=AF.Sqrt, bias=eps_t[:], scale=1.0)
    nc.vector.reciprocal(out=var, in_=var)
    nc.vector.tensor_scalar(out=out_dst, in0=z, scalar1=mean, scalar2=var,
                            op0=ALU.subtract, op1=ALU.mult)
    nc.vector.tensor_tensor(out=out_dst, in0=out_dst, in1=g[:, 0:d], op=ALU.mult)
    nc.vector.tensor_tensor(out=out_dst, in0=out_dst, in1=b[:, 0:d], op=ALU.add)


# ============================================================
# host-side sharding + entry point
# ============================================================

def shard_inputs(inputs, c: Cfg = None):
    c = c or Cfg()
    w = np.asarray(inputs["w"], np.float32)
    r = np.asarray(inputs["r"], np.float32)
    mems = np.asarray(inputs["mems"], np.float32)
    qkv_w = np.asarray(inputs["qkv_w"], np.float32)
    r_net_w = np.asarray(inputs["r_net_w"], np.float32)
    o_w = np.asarray(inputs["o_w"], np.float32)
    r_w_bias = np.asarray(inputs["r_w_bias"], np.float32).reshape(-1)
    r_r_bias = np.asarray(inputs["r_r_bias"], np.float32).reshape(-1)
    NHD = qkv_w.shape[1] // 3
    rt = np.ascontiguousarray(r[:, 0, :].T)
    in_maps = []
    for core in range(c.N_CORES):
        b, hh = core // 2, core % 2
        hsl = slice(hh * c.HD, (hh + 1) * c.HD)
        xt_c = np.concatenate([mems[:, b, :], w[:, b, :]], axis=0).T
        qkvw_c = np.concatenate([qkv_w[:, j * NHD + hh * c.HD:
                                       j * NHD + (hh + 1) * c.HD]
                                 for j in range(3)], axis=1)
        in_maps.append({
            "xt": np.ascontiguousarray(xt_c),
            "rt": rt,
            "qkvw": np.ascontiguousarray(qkvw_c),
            "rnetw": np.ascontiguousarray(r_net_w[:, hsl]),
            "oww": np.ascontiguousarray(o_w[hsl, :]),
            "rwb": np.ascontiguousarray(r_w_bias[hsl][None, :]),
            "rrb": np.ascontiguousarray(r_r_bias[hsl][None, :]),
            "ln1g": np.asarray(inputs["ln1_g"], np.float32).reshape(1, -1),
            "ln1b": np.asarray(inputs["ln1_b"], np.float32).reshape(1, -1),
            "ln2g": np.asarray(inputs["ln2_g"], np.float32).reshape(1, -1),
            "ln2b": np.asarray(inputs["ln2_b"], np.float32).reshape(1, -1),
            "ffw1": np.asarray(inputs["ff_w1"], np.float32),
            "ffb1": np.asarray(inputs["ff_b1"], np.float32).reshape(1, -1),
            "ffw2": np.asarray(inputs["ff_w2"], np.float32),
            "ffb2": np.asarray(inputs["ff_b2"], np.float32).reshape(1, -1),
            "wres": np.ascontiguousarray(w[hh * c.TOKF:(hh + 1) * c.TOKF, b, :]),
        })
    return in_maps


def unshard_output(results, inputs, c: Cfg = None):
    c = c or Cfg()
    w = np.asarray(inputs["w"])
    Q, B, D = w.shape
    out = np.zeros((Q, B, D), np.float32)
    for core in range(c.N_CORES):
        b, hh = core // 2, core % 2
        out[hh * c.TOKF:(hh + 1) * c.TOKF, b, :] = results[core]["out"]
    return out


_NC_CACHE = {}


def kernel(**inputs):
    if "nc" not in _NC_CACHE:
        _NC_CACHE["nc"] = build_kernel()
    nc = _NC_CACHE["nc"]
    in_maps = shard_inputs(inputs)
    from concourse.bass_utils import run_bass_kernel_spmd
    res = run_bass_kernel_spmd(nc, in_maps, core_ids=list(range(Cfg.N_CORES)))
    return unshard_output(res.results, inputs)
